# revision 43
# baseline (speedup 1.0000x reference)
"""DeepSeek MLA head — Trainium2 Bass kernel, 8 NeuronCores.

Sharding: 8 cores = 2 batches x 4 cores. Core c: batch b=c//4, j=c%4 owns
token supertile j (512 tokens) for all latent/q_b compute, heads [4j,4j+4)
for attention, and hid output rows [512j,512j+512) for o_proj.

Phases:
  P1   each core computes q/kv low-rank latents + RMSNorm + shared k_pe rope
       for ITS supertile; kv latents are AllGathered (GKV) early.
  QB   q_b + rope computed LOCALLY for ALL 16 heads on my 512 tokens (same
       PE cols as 4 heads x 2048 tokens), then one AllToAll exchanges
       [my tokens, all heads] -> [all tokens, my heads].  This removes the
       big q-latent AllGather from the critical path: the exchange happens
       after local compute instead of blocking it.
  P1b  per-head k_nope/V up-projections from the gathered kv latents.
  P2   attention per head: scores / exp / AV, 2-chunk [128,1024] PSUM
       groups; softmax denominators accumulated with DVE f16 adds + one
       ones-matmul per (head, q-supertile). exp shifted by -6. Causal mask
       multiplies restricted to the 128-wide diagonal triangle blocks
       (ranged accumulates skip the stale trimmed columns).
  P3   per q-supertile: LOCAL partial o_proj over my 4 heads producing
       out^T [2048 hid, 512 tok] f16 partials, then ReduceScatter(add)
       writes each core's 512 hid rows directly into the output tensor.
       The o_proj compute hides inside P2 of later supertiles and the tail
       is only the last ReduceScatter.

Layout: activations transposed [feature, token] on-chip. Host folds RMSNorm
gains + DeepSeek RoPE interleave permutation into the weights, and the
rotate-half SIGN into the sin table (rows 64:96 negated) so rotate-half is
two plain partition-shifted copies instead of a PE matmul. Matmul operands
f16, f32 PSUM. Input DMAs are spread across engine queues so the startup is
bandwidth- not queue-limited.
"""
import sys
import types

sys.path.insert(0, "/opt/trn_rl_repo")

import numpy as np

B, S, HID, NH = 2, 2048, 2048, 16
ROPE, NOPE, VDIM = 64, 64, 128
QHEAD, QLORA, KVLORA = 128, 682, 256
THETA = 128000.0
SCALE = 1.0 / float(np.sqrt(128.0))
EPS = 1e-6
NEGC = -6.0          # exp shift: exp(SCALE*s - 6), softmax-invariant
HPC = 4              # heads per core (attention)
NCORES = 8
QCH = [128, 128, 128, 128, 128, 42]   # qlora partition chunks
NST = 4              # 512-token supertiles per batch
STW = 512
GROUPS = [[0, 1, 2, 3], [4, 5, 6, 7]]

_PROGRAM = None


def _ensure_axon_hooks_shim():
    if "antenv.axon_hooks" in sys.modules:
        return
    try:
        from trn_agent_boot.trn_boot import _ntff_profile_via_ctypes
        hook = _ntff_profile_via_ctypes("/opt/axon/libaxon_pjrt.so")
    except Exception:
        hook = None
    m = types.ModuleType("antenv.axon_hooks")
    m.get_axon_ntff_profile_hook = lambda: hook
    m.set_axon_ntff_profile_hook = lambda h: None
    sys.modules["antenv.axon_hooks"] = m


def _build_program():
    import concourse.bass as bass  # noqa: F401
    import concourse.mybir as mybir
    import concourse.tile as tile
    from concourse import bacc

    f16 = mybir.dt.float16
    f32 = mybir.dt.float32
    AF = mybir.ActivationFunctionType

    nc = bacc.Bacc("TRN2", target_bir_lowering=False, debug=False,
                   num_devices=NCORES)
    for cv in (EPS, NEGC):
        t = nc.alloc_sbuf_tensor(f"const-{cv}", [128, 1], f32)
        nc.gpsimd.memset(t.ap(), cv)
        nc.const_aps.aps[(f32, cv)] = t.ap()
    nc.all_engine_barrier()

    def din(name, shape, dt=f16):
        return nc.dram_tensor(name, shape, dt, kind="ExternalInput").ap()

    xT = din("xT", [HID, STW])            # my supertile of x, transposed
    waq = din("waq", [HID, QLORA])        # q_a_w
    wakv = din("wakv", [HID, 320])        # kv_a_w cols: [ckv 256 | kpe-perm 64]
    wqb = din("wqb", [QLORA, HPC * 128])  # MY heads: [nope64|pe64-perm], ln folded
    wkn = din("wkn", [KVLORA, HPC * 64])  # my heads: knope cols, ln folded
    wv = din("wv", [KVLORA, HPC * 128])   # my heads: v cols, ln folded
    wo = din("wo", [HPC * VDIM, HID])     # MY heads' o_w rows x ALL hid cols
    cosT = din("cosT", [128, S])          # rows 0:64 = 1, rows 64:128 = cos
    sinT = din("sinT", [128, S])          # rows 0:64=0, 64:96=-sin[0:32], 96:128=sin[32:64]
    cosM = din("cosM", [64, STW])         # my supertile slice, rows 64:128 of cosT
    sinM = din("sinM", [64, STW])         # my supertile slice, rows 64:128 of sinT
    maskT = din("maskT", [128, 896])      # shifted causal window: m[k,c]=(k<=c-384)
    # out^T: rows = qs-major x (my 512 hid), cols = 512 toks of that qs
    out = nc.dram_tensor("out", [NST * STW, STW], f16, kind="ExternalOutput").ap()

    from contextlib import ExitStack
    with tile.TileContext(nc) as tc, ExitStack() as ctx:
        const = ctx.enter_context(tc.tile_pool(name="const", bufs=1))
        waqp = ctx.enter_context(tc.tile_pool(name="waqp", bufs=16))
        wakvp = ctx.enter_context(tc.tile_pool(name="wakvp", bufs=16))
        xtwo = ctx.enter_context(tc.tile_pool(name="xtwo", bufs=16))
        qrawp = ctx.enter_context(tc.tile_pool(name="qrawp", bufs=6))
        lrsp = ctx.enter_context(tc.tile_pool(name="lrsp", bufs=2))
        wop = ctx.enter_context(tc.tile_pool(name="wop", bufs=HPC))
        latkp = ctx.enter_context(tc.tile_pool(name="latkp", bufs=7))
        rawp = ctx.enter_context(tc.tile_pool(name="rawp", bufs=1))
        sqp = ctx.enter_context(tc.tile_pool(name="sqp", bufs=1))
        smallp = ctx.enter_context(tc.tile_pool(name="smallp", bufs=2))
        bcp = ctx.enter_context(tc.tile_pool(name="bcp", bufs=2))
        scr1 = ctx.enter_context(tc.tile_pool(name="scr1", bufs=1))
        persist = ctx.enter_context(tc.tile_pool(name="persist", bufs=HPC))
        ptp = ctx.enter_context(tc.tile_pool(name="ptp", bufs=2))
        accp = ctx.enter_context(tc.tile_pool(name="accp", bufs=4))
        aoutp = ctx.enter_context(tc.tile_pool(name="aoutp", bufs=8))
        dramp = ctx.enter_context(tc.tile_pool(name="dram", bufs=1, space="DRAM"))
        # PSUM: 8 banks = big 2x[128,1024] (4) + out 3x[128,512] (3) + misc 1
        ps_big = ctx.enter_context(tc.tile_pool(name="ps_big", bufs=2, space="PSUM"))
        ps_out = ctx.enter_context(tc.tile_pool(name="ps_out", bufs=3, space="PSUM"))
        ps_misc = ctx.enter_context(tc.tile_pool(name="ps_misc", bufs=1, space="PSUM"))

        # ---- constants / weights into SBUF, spread across engine queues ----
        sb_waq = [waqp.tile([128, QLORA], f16, tag="waq", name=f"waq{hc}")
                  for hc in range(16)]
        sb_wakv = [wakvp.tile([128, 320], f16, tag="wakv", name=f"wakv{hc}")
                   for hc in range(16)]
        W = HPC * 128
        sb_wqb = const.tile([128, 6 * W], f16, tag="wqb")
        sb_wkn = const.tile([128, 2 * HPC * 64], f16, tag="wkn")
        sb_wv = const.tile([128, 2 * HPC * 128], f16, tag="wv")
        sb_cos = const.tile([128, S], f16, tag="cos")
        sb_sin = const.tile([128, S], f16, tag="sin")
        sb_cosM = const.tile([64, STW], f16, tag="cosM")
        sb_sinM = const.tile([64, STW], f16, tag="sinM")
        sb_mask = const.tile([128, 896], f16, tag="mask")
        sb_ones = const.tile([128, 1], f16, tag="ones")
        sb_onesr = const.tile([1, 128], f16, tag="onesr")

        qoff = [0, 128, 256, 384, 512, 640]
        xt = [xtwo.tile([128, STW], f16, tag="xt", name=f"xt{hc}")
              for hc in range(16)]
        # sync queue: xt (needed first, for kv latents)
        for hc in range(16):
            nc.sync.dma_start(out=xt[hc][:], in_=xT[hc * 128:(hc + 1) * 128, :])
        # scalar queue: wakv (needed first), then waq (q latents)
        for hc in range(16):
            nc.scalar.dma_start(out=sb_wakv[hc][:],
                                in_=wakv[hc * 128:(hc + 1) * 128, :])
        for hc in range(16):
            nc.scalar.dma_start(out=sb_waq[hc][:],
                                in_=waq[hc * 128:(hc + 1) * 128, :])
        # gpsimd queue: rope tables, mask, then wqb
        nc.gpsimd.dma_start(out=sb_cosM[:], in_=cosM[:])
        nc.gpsimd.dma_start(out=sb_sinM[:], in_=sinM[:])
        nc.gpsimd.dma_start(out=sb_cos[:], in_=cosT[:])
        nc.gpsimd.dma_start(out=sb_sin[:], in_=sinT[:])
        nc.gpsimd.dma_start(out=sb_mask[:], in_=maskT[:])
        for c in range(6):
            nc.gpsimd.dma_start(out=sb_wqb[:QCH[c], c * W:(c + 1) * W],
                                in_=wqb[qoff[c]:qoff[c] + QCH[c], :])
        for c in range(2):
            nc.gpsimd.dma_start(out=sb_wkn[:, c * HPC * 64:(c + 1) * HPC * 64],
                                in_=wkn[c * 128:(c + 1) * 128, :])
            nc.gpsimd.dma_start(out=sb_wv[:, c * HPC * 128:(c + 1) * HPC * 128],
                                in_=wv[c * 128:(c + 1) * 128, :])
        nc.vector.memset(sb_ones[:], 1.0)
        nc.vector.memset(sb_onesr[:], 1.0)
        # wo loads on sync queue behind xt
        sb_wo = []
        for h in range(HPC):
            t = wop.tile([128, HID], f16, tag="wo", name=f"wo{h}")
            nc.sync.dma_start(out=t[:], in_=wo[h * 128:(h + 1) * 128, :])
            sb_wo.append(t)

        # persistent per-head tensors (my 4 heads x all tokens)
        qfT = [persist.tile([128, S], f16, tag="qf", name=f"qfT{h}") for h in range(HPC)]
        kfT = [persist.tile([128, S], f16, tag="kf", name=f"kfT{h}") for h in range(HPC)]
        VT = [persist.tile([128, 16 * VDIM], f16, tag="vh", name=f"VT{h}") for h in range(HPC)]

        # DRAM bounce buffers
        bgk_in = dramp.tile([128, 1280], f16, name="bgk_in", tag="bgk_in")
        bgk_out = dramp.tile([4 * 128, 1280], f16, name="bgk_out", tag="bgk_out")
        bgq_in = dramp.tile([128, 3584], f16, name="bgq_in", tag="bgq_in")
        bgq_out = dramp.tile([4 * 128, 3584], f16, name="bgq_out", tag="bgq_out")

        # ================= P1: latents for MY supertile =================
        sums = ps_out.tile([64, STW], f32, tag="out", name="sums")  # row0 q, row32 k

        # kv latents: ckv (2 chunk halves) + kpe (64 rows), then GKV gather
        psk = ps_big.tile([128, 1024], f32, tag="big", name="klat")
        for half in range(2):
            for hc in range(16):
                nc.tensor.matmul(
                    psk[:, half * STW:(half + 1) * STW],
                    sb_wakv[hc][:, half * 128:(half + 1) * 128],
                    xt[hc][:],
                    start=(hc == 0), stop=(hc == 15))
        kraw = rawp.tile([128, 1024], f16, tag="rawk")
        nc.vector.tensor_copy(kraw[:], psk[:])
        sqk = sqp.tile([128, 1024], f16, tag="sq")
        nc.scalar.activation(sqk[:], psk[:], AF.Square)
        for half in range(2):
            nc.tensor.matmul(sums[32:33, :], sb_ones[:, :],
                             sqk[:, half * STW:(half + 1) * STW],
                             start=(half == 0), stop=(half == 1))

        psp = ps_misc.tile([128, STW], f32, tag="misc", name="kpelat")
        for hc in range(16):
            nc.tensor.matmul(psp[0:64, :], sb_wakv[hc][:, 256:320], xt[hc][:],
                             start=(hc == 0), stop=(hc == 15))
        kperaw = rawp.tile([64, STW], f16, tag="kpe")
        nc.vector.tensor_copy(kperaw[:], psp[0:64, :])

        # rstd for k, normalize kraw
        stdk = smallp.tile([1, STW], f32, tag="stdk", bufs=1)
        nc.scalar.activation(stdk[:], sums[32:33, :], AF.Sqrt,
                             bias=EPS, scale=1.0 / KVLORA)
        rstdkf = smallp.tile([1, STW], f32, tag="rstdkf", bufs=1)
        nc.vector.reciprocal_approx_fast(out=rstdkf[:], in_=stdk[:])
        rstdk = smallp.tile([1, STW], f16, tag="rstdk", bufs=1)
        nc.vector.tensor_copy(rstdk[:], rstdkf[:])
        bck = ps_misc.tile([128, STW], f32, tag="misc", name="bck")
        nc.tensor.matmul(bck[:], sb_onesr[:], rstdk[:1, :], start=True, stop=True)
        bcks = bcp.tile([128, STW], f16, tag="bc", name="bcks")
        nc.vector.tensor_copy(bcks[:], bck[:])
        nc.vector.tensor_mul(kraw[:, 0:STW], kraw[:, 0:STW], bcks[:])
        nc.vector.tensor_mul(kraw[:, STW:1024], kraw[:, STW:1024], bcks[:])

        # shared k_pe rope on [64, 512]: rotate-half = 2 plain swapped copies
        # (sign folded into sinM rows 64:96); tables rows 64:128.
        rotk = scr1.tile([64, STW], f16, tag="rotk")
        nc.scalar.copy(out=rotk[0:32, :], in_=kperaw[32:64, :])
        nc.scalar.copy(out=rotk[32:64, :], in_=kperaw[0:32, :])
        t1k = scr1.tile([64, STW], f16, tag="t1k")
        nc.vector.tensor_mul(t1k[:], rotk[:], sb_sinM[:])
        t2k = scr1.tile([64, STW], f16, tag="t2k")
        nc.vector.tensor_mul(t2k[:], kperaw[:], sb_cosM[:])
        kpero = kperaw
        nc.vector.tensor_add(kpero[:], t1k[:], t2k[:])

        nc.gpsimd.dma_start(bgk_in[:, 0:1024], kraw[:])
        # fold [64,512] -> [128,256] (row k -> rows 2k,2k+1)
        nc.gpsimd.dma_start(bgk_in[:, 1024:1280], kpero[:])
        nc.gpsimd.collective_compute(
            "AllGather", mybir.AluOpType.bypass, replica_groups=GROUPS,
            ins=[bgk_in.opt()], outs=[bgk_out.opt()])

        # q latents: 3 chunk-pairs, gathered RAW (rstd folded post-gather)
        for p in range(3):
            ps = ps_big.tile([128, 1024], f32, tag="big", name=f"qlat{p}")
            for half in range(2):
                c = 2 * p + half
                for hc in range(16):
                    nc.tensor.matmul(
                        ps[:QCH[c], half * STW:(half + 1) * STW],
                        sb_waq[hc][:, qoff[c]:qoff[c] + QCH[c]],
                        xt[hc][:],
                        start=(hc == 0), stop=(hc == 15))
            raw = qrawp.tile([128, 1024], f16, tag="rawq", name=f"rawq{p}",
                             bufs=6)
            nc.vector.tensor_copy(raw[:], ps[:])
            sq = sqp.tile([128, 1024], f16, tag="sq")
            nc.scalar.activation(sq[:], ps[:], AF.Square)
            for half in range(2):
                c = 2 * p + half
                nc.tensor.matmul(sums[0:1, :], sb_ones[:QCH[c], :],
                                 sq[:QCH[c], half * STW:(half + 1) * STW],
                                 start=(c == 0), stop=(c == 5))
            nc.gpsimd.dma_start(bgq_in[:, p * 1024:(p + 1) * 1024], raw[:])

        # rstd for q, folded [1,512]->[64,8] into the gather payload
        stdq = smallp.tile([1, STW], f32, tag="stdq", bufs=1)
        nc.scalar.activation(stdq[:], sums[0:1, :], AF.Sqrt,
                             bias=EPS, scale=1.0 / QLORA)
        rstdqf = smallp.tile([1, STW], f32, tag="rstdqf", bufs=1)
        nc.vector.reciprocal_approx_fast(out=rstdqf[:], in_=stdq[:])
        rstdq = smallp.tile([1, STW], f16, tag="rstdq", bufs=1)
        nc.vector.tensor_copy(rstdq[:], rstdqf[:])
        nc.gpsimd.dma_start(bgq_in[0:1, 3072:3584], rstdq[:])
        nc.gpsimd.collective_compute(
            "AllGather", mybir.AluOpType.bypass, replica_groups=GROUPS,
            ins=[bgq_in.opt()], outs=[bgq_out.opt()])

        # q_b + rope for one gathered supertile
        def qb_block(st):
            cols = slice(st * STW, (st + 1) * STW)
            gq = []
            for p in range(3):
                t = qrawp.tile([128, 1024], f16, tag="rawq",
                               name=f"gq{st}_{p}", bufs=6)
                nc.sync.dma_start(
                    t[:], bgq_out[st * 128:(st + 1) * 128,
                                  p * 1024:(p + 1) * 1024])
                gq.append(t)
            grs = lrsp.tile([1, STW], f16, tag="lrs", name=f"grs{st}", bufs=2)
            nc.sync.dma_start(grs[:],
                              bgq_out[st * 128:st * 128 + 1, 3072:3584])

            bcq = ps_misc.tile([128, STW], f32, tag="misc", name=f"bcq{st}")
            nc.tensor.matmul(bcq[:], sb_onesr[:], grs[:1, :], start=True, stop=True)
            bcqs = bcp.tile([128, STW], f16, tag="bc", name=f"bcqs{st}")
            nc.scalar.copy(out=bcqs[:], in_=bcq[:])
            sinq = scr1.tile([128, STW], f16, tag="sinq", name=f"sinq{st}", bufs=2)
            cosq = scr1.tile([128, STW], f16, tag="cosq", name=f"cosq{st}", bufs=2)
            nc.vector.tensor_mul(sinq[:], sb_sin[:, cols], bcqs[:])
            nc.vector.tensor_mul(cosq[:], sb_cos[:, cols], bcqs[:])

            for pr in range(2):
                qra = scr1.tile([128, 2 * STW], f16, tag="qra",
                                name=f"qra{st}_{pr}", bufs=1)
                for i in range(2):
                    h = 2 * pr + i
                    psq = ps_out.tile([128, STW], f32, tag="out",
                                      name=f"psq{st}_{h}")
                    for c in range(6):
                        nc.tensor.matmul(
                            psq[:],
                            sb_wqb[:QCH[c], c * W + h * 128:c * W + (h + 1) * 128],
                            gq[c // 2][:QCH[c], (c % 2) * STW:(c % 2 + 1) * STW],
                            start=(c == 0), stop=(c == 5))
                    nc.vector.tensor_copy(qra[:, i * STW:(i + 1) * STW], psq[:])
                # rope: rows 0:64 = qra*cosq (nope, rstd); 64:128 full rope,
                # rotate-half = 2 plain swapped copies (sign in sin table)
                for i in range(2):
                    h = 2 * pr + i
                    hs = slice(i * STW, (i + 1) * STW)
                    rotc = scr1.tile([128, STW], f16, tag="rotc",
                                     name=f"rotc{st}_{pr}_{i}", bufs=2)
                    nc.scalar.copy(out=rotc[64:96, :], in_=qra[96:128, hs])
                    nc.scalar.copy(out=rotc[96:128, :], in_=qra[64:96, hs])
                    nc.vector.tensor_mul(qfT[h][0:64, cols],
                                         qra[0:64, hs], cosq[0:64, :])
                    nc.vector.tensor_mul(rotc[64:128, :], rotc[64:128, :],
                                         sinq[64:128, :])
                    nc.vector.tensor_mul(qra[64:128, hs], qra[64:128, hs],
                                         cosq[64:128, :])
                    nc.vector.tensor_add(qfT[h][64:128, cols],
                                         rotc[64:128, :], qra[64:128, hs])

        # ================= P1b: kn/V over all supertiles (needs GKV) =======
        for st in range(NST):
            cols = slice(st * STW, (st + 1) * STW)
            gk = [latkp.tile([128, STW], f16, tag="latk", name=f"gk{st}_{c}")
                  for c in range(2)]
            for c in range(2):
                nc.sync.dma_start(
                    gk[c][:],
                    bgk_out[st * 128:(st + 1) * 128, c * STW:(c + 1) * STW])
            # shared k_pe: unfold [128,256] -> [64,512] straight into each
            # head's kfT rows 64:128
            for h in range(HPC):
                eng = nc.scalar if h % 2 == 0 else nc.gpsimd
                eng.dma_start(
                    out=kfT[h][64:128, cols],
                    in_=bgk_out[st * 128:(st + 1) * 128, 1024:1280])
            for hp in range(2):
                pskn = ps_out.tile([128, STW], f32, tag="out", name=f"kn{st}_{hp}")
                for c in range(2):
                    nc.tensor.matmul(
                        pskn[:],
                        sb_wkn[:, c * HPC * 64 + hp * 128:c * HPC * 64 + (hp + 1) * 128],
                        gk[c][:],
                        start=(c == 0), stop=(c == 1))
                nc.scalar.copy(out=kfT[2 * hp][0:64, cols], in_=pskn[0:64, :])
                nc.scalar.copy(out=kfT[2 * hp + 1][0:64, cols], in_=pskn[64:128, :])
            for h in range(HPC):
                psv = ps_out.tile([128, STW], f32, tag="out", name=f"psv{st}_{h}")
                for tcn in range(4):
                    for c in range(2):
                        nc.tensor.matmul(
                            psv[:, tcn * VDIM:(tcn + 1) * VDIM],
                            gk[c][:, tcn * 128:(tcn + 1) * 128],
                            sb_wv[:, c * HPC * 128 + h * 128:c * HPC * 128 + (h + 1) * 128],
                            start=(c == 0), stop=(c == 1))
                nc.vector.tensor_copy(VT[h][:, st * STW:(st + 1) * STW], psv[:])

        # ================= P2 / P3, per q-supertile ========================
        # pre-zero pt slots: qs==0 diagonal chunks use full-width mask muls
        # that must see finite values in the stale trimmed columns
        for z in range(2):
            ptz = ptp.tile([128, 1024], f16, tag="pt", name=f"ptz{z}")
            nc.vector.memset(ptz[:], 0.0)

        def norm(qs, h, ssums, accs, aouts):
            bca = ps_misc.tile([128, STW], f32, tag="misc", name=f"bca{qs}_{h}")
            nc.tensor.matmul(bca[:], sb_onesr[:], ssums[h][:1, :],
                             start=True, stop=True)
            bcas = bcp.tile([128, STW], f16, tag="bc", name=f"bcas{qs}_{h}")
            nc.scalar.copy(out=bcas[:], in_=bca[:])
            ao = aoutp.tile([128, STW], f16, tag="aout", name=f"ao{qs}_{h}")
            nc.vector.tensor_mul(ao[:], accs[h][:], bcas[:])
            aouts.append(ao)

        def p2_block(qs):
            nkc = 4 * qs + 4
            ng = nkc // 2
            accs, ssums, aouts = [], [], []
            kcs = (list(range(4 * qs, nkc)) + list(range(0, 4 * qs))
                   if qs > 0 else list(range(nkc)))
            for h in range(HPC):
                outT = ps_out.tile([128, STW], f32, tag="out", name=f"oT{qs}_{h}")
                # two partial prob-sum accumulators: even chunks chain on
                # DVE (accA), odd chunks on GpSimd (accB); the combine is
                # folded into the ssum PSUM accumulation
                accA = accp.tile([128, STW], f16, tag="acc", name=f"accA{qs}_{h}")
                accB = accp.tile([128, STW], f16, tag="acc", name=f"accB{qs}_{h}")

                stps, pts = {}, {}
                def sc(g):
                    stp = ps_big.tile([128, 1024], f32, tag="big",
                                      name=f"sc{qs}_{h}_{g}")
                    trim = [0, 0]
                    for half in range(2):
                        kc = kcs[2 * g + half]
                        j = kc - 4 * qs
                        tr = 128 * j if j > 0 else 0
                        trim[half] = tr
                        nc.tensor.matmul(
                            stp[:, half * STW + tr:(half + 1) * STW],
                            kfT[h][:, kc * 128:(kc + 1) * 128],
                            qfT[h][:, qs * STW + tr:(qs + 1) * STW],
                            start=True, stop=True)
                    pt = ptp.tile([128, 1024], f16, tag="pt")
                    if trim[0] == 0 and trim[1] == 0:
                        nc.scalar.activation(pt[:], stp[:], AF.Exp,
                                             bias=NEGC, scale=SCALE)
                    else:
                        for half in range(2):
                            tr = trim[half]
                            nc.scalar.activation(
                                pt[:, half * STW + tr:(half + 1) * STW],
                                stp[:, half * STW + tr:(half + 1) * STW],
                                AF.Exp, bias=NEGC, scale=SCALE)
                    for half in range(2):
                        kc = kcs[2 * g + half]
                        j = kc - 4 * qs
                        if j >= 0:
                            tr = 128 * j
                            if qs == 0 and j > 0:
                                # full-width mask (zeroes stale cols too)
                                ph = pt[:, half * STW:(half + 1) * STW]
                                nc.vector.tensor_mul(
                                    ph, ph,
                                    sb_mask[:, 384 - tr:896 - tr])
                            else:
                                # triangle block only
                                blk = slice(half * STW + tr,
                                            half * STW + tr + 128)
                                nc.vector.tensor_mul(
                                    pt[:, blk], pt[:, blk],
                                    sb_mask[:, 384:512])
                    pts[g] = pt

                def av(g, first):
                    pt = pts.pop(g)
                    for half in range(2):
                        idx = 2 * g + half
                        kc = kcs[idx]
                        j = kc - 4 * qs
                        tr = 128 * j if (j > 0 and qs > 0) else 0
                        ph = pt[:, half * STW + tr:(half + 1) * STW]
                        nc.tensor.matmul(outT[:, tr:STW],
                                         VT[h][:, kc * VDIM:(kc + 1) * VDIM],
                                         ph,
                                         start=(idx == 0), stop=(idx == nkc - 1))
                        ph2 = pt[:, half * STW + tr:(half + 1) * STW]
                        if idx == 0:
                            first[0] = pt[:, half * STW:(half + 1) * STW]
                        elif idx == 1:
                            if tr > 0:
                                nc.gpsimd.memset(accB[:, 0:tr], 0.0)
                                nc.gpsimd.tensor_copy(accB[:, tr:STW], ph2)
                            else:
                                nc.gpsimd.tensor_copy(
                                    accB[:], pt[:, half * STW:(half + 1) * STW])
                        elif idx == 2:
                            if tr > 0:
                                nc.vector.tensor_copy(accA[:, 0:tr],
                                                      first[0][:, 0:tr])
                            nc.vector.tensor_add(
                                accA[:, tr:STW], first[0][:, tr:STW], ph2)
                        elif idx % 2 == 0:
                            nc.vector.tensor_add(
                                accA[:, tr:STW], accA[:, tr:STW], ph2)
                        else:
                            nc.gpsimd.tensor_add(
                                accB[:, tr:STW], accB[:, tr:STW], ph2)

                first = [None]
                sc(0)
                if h >= 1:
                    norm(qs, h - 1, ssums, accs, aouts)
                for g in range(ng):
                    if g + 1 < ng:
                        sc(g + 1)
                    av(g, first)
                ssum = ps_misc.tile([1, STW], f32, tag="misc", name=f"ss{qs}_{h}")
                nc.tensor.matmul(ssum[:], sb_ones[:, :], accA[:],
                                 start=True, stop=False)
                nc.tensor.matmul(ssum[:], sb_ones[:, :], accB[:],
                                 start=False, stop=True)
                rsf = smallp.tile([1, STW], f32, tag="rsf", bufs=2)
                nc.vector.reciprocal_approx_fast(out=rsf[:], in_=ssum[:])
                rs = smallp.tile([1, STW], f16, tag="rs", bufs=4)
                nc.vector.tensor_copy(rs[:], rsf[:])
                ssums.append(rs)
                accs.append(outT)
            norm(qs, HPC - 1, ssums, accs, aouts)
            return aouts

        def p3_block(qs, aouts):
            # local partial o_proj: out^T [2048 hid, 512 tok] over my 4 heads
            rs_in = dramp.tile([NH * 128, STW], f16, tag="rs_in",
                               name=f"rsin{qs}", bufs=2)
            for hcn in range(16):
                pso = ps_out.tile([128, STW], f32, tag="out", name=f"pso{qs}_{hcn}")
                for h in range(HPC):
                    nc.tensor.matmul(
                        pso[:],
                        sb_wo[h][:, hcn * 128:(hcn + 1) * 128],
                        aouts[h][:],
                        start=(h == 0), stop=(h == HPC - 1))
                ob = waqp.tile([128, STW], f16, tag="waq", name=f"ob{qs}_{hcn}")
                if hcn % 2 == 0:
                    nc.scalar.copy(out=ob[:], in_=pso[:])
                else:
                    nc.vector.tensor_copy(ob[:], pso[:])
                deng = nc.sync if hcn % 2 == 0 else nc.gpsimd
                deng.dma_start(rs_in[hcn * 128:(hcn + 1) * 128, :], ob[:])
            rs_out = dramp.tile([STW, STW], f16, tag="rs_out",
                                name=f"rsout{qs}", bufs=2)
            nc.gpsimd.collective_compute(
                "ReduceScatter", mybir.AluOpType.add, replica_groups=GROUPS,
                ins=[rs_in.opt()], outs=[rs_out.opt()])
            nc.sync.dma_start(out[qs * STW:(qs + 1) * STW, :], rs_out[:])

        for st in range(NST):
            qb_block(st)
        for qs in range(NST):
            aouts = p2_block(qs)
            p3_block(qs, aouts)

    nc.compile()
    return nc


def _host_prep(inputs):
    f16 = np.float16
    x = np.asarray(inputs["x"], np.float32)
    q_a_w = np.asarray(inputs["q_a_w"], np.float32)
    q_a_ln = np.asarray(inputs["q_a_ln_w"], np.float32)
    q_b_w = np.asarray(inputs["q_b_w"], np.float32)
    kv_a_w = np.asarray(inputs["kv_a_w"], np.float32)
    kv_a_ln = np.asarray(inputs["kv_a_ln_w"], np.float32)
    kv_b_w = np.asarray(inputs["kv_b_w"], np.float32)
    o_w = np.asarray(inputs["o_w"], np.float32)

    perm = np.concatenate([np.arange(0, ROPE, 2), np.arange(1, ROPE, 2)])
    q_b_f = q_b_w * q_a_ln[:, None]
    kv_b_f = kv_b_w * kv_a_ln[:, None]

    # kv_a: [ckv 256 | kpe perm 64]
    wakv = np.concatenate(
        [kv_a_w[:, :KVLORA], kv_a_w[:, KVLORA:][:, perm]], axis=1).astype(f16)
    waq = q_a_w.astype(f16)

    # rope tables (transposed [dim, pos]); rotate-half sign folded into sin:
    # rows 64:96 = -sin[0:32], rows 96:128 = +sin[32:64]
    inv = 1.0 / (THETA ** (np.arange(0, ROPE, 2, dtype=np.float64) / ROPE))
    freqs = np.outer(np.arange(S, dtype=np.float64), inv)      # [S, 32]
    cos64 = np.concatenate([np.cos(freqs), np.cos(freqs)], -1).T  # [64, S]
    sin64 = np.concatenate([np.sin(freqs), np.sin(freqs)], -1).T
    cosT = np.concatenate([np.ones((64, S)), cos64], 0).astype(f16)
    sinT = np.concatenate([np.zeros((64, S)), -sin64[0:32], sin64[32:64]],
                          0).astype(f16)

    # shifted causal window: maskT[k, c] = k <= c - 384; cols 384:512 are the
    # plain [128,128] triangle k <= q'
    k_i = np.arange(128)[:, None]
    c_i = np.arange(896)[None, :]
    maskT = (k_i <= c_i - 384).astype(f16)

    in_maps = []
    for core in range(NCORES):
        b = core // 4
        j = core % 4
        heads = [HPC * j + i for i in range(HPC)]
        wqb = np.concatenate(
            [np.concatenate(
                [q_b_f[:, h * QHEAD:h * QHEAD + NOPE],
                 q_b_f[:, h * QHEAD + NOPE:(h + 1) * QHEAD][:, perm]], 1)
             for h in heads], axis=1).astype(f16)
        wkn = np.concatenate(
            [kv_b_f[:, h * (NOPE + VDIM):h * (NOPE + VDIM) + NOPE]
             for h in heads], axis=1).astype(f16)
        wv = np.concatenate(
            [kv_b_f[:, h * (NOPE + VDIM) + NOPE:(h + 1) * (NOPE + VDIM)]
             for h in heads], axis=1).astype(f16)
        wo = np.concatenate(
            [o_w[h * VDIM:(h + 1) * VDIM, :] for h in heads], axis=0).astype(f16)
        scols = slice(j * STW, (j + 1) * STW)
        in_maps.append({
            "xT": np.ascontiguousarray(x[b].T[:, scols]).astype(f16),
            "waq": waq, "wakv": wakv, "wqb": wqb, "wkn": wkn, "wv": wv,
            "wo": wo, "cosT": cosT, "sinT": sinT,
            "cosM": np.ascontiguousarray(cosT[64:128, scols]),
            "sinM": np.ascontiguousarray(sinT[64:128, scols]),
            "maskT": maskT,
        })
    return in_maps


def kernel(**inputs):
    global _PROGRAM
    _ensure_axon_hooks_shim()
    from concourse.bass_utils import run_bass_kernel_spmd

    if _PROGRAM is None:
        _PROGRAM = _build_program()
    in_maps = _host_prep(inputs)
    res = run_bass_kernel_spmd(_PROGRAM, in_maps, list(range(NCORES)))
    out = np.zeros((B, S, HID), np.float32)
    for core in range(NCORES):
        b, j = core // 4, core % 4
        r = res.results[core]["out"].astype(np.float32)   # [4*512 hid-qs, 512]
        for qs in range(NST):
            out[b][qs * STW:(qs + 1) * STW, j * STW:(j + 1) * STW] = \
                r[qs * STW:(qs + 1) * STW, :].T
    return out


# revision 44
# speedup vs baseline: 1.0087x; 1.0087x over previous
"""DeepSeek MLA head — Trainium2 Bass kernel, 8 NeuronCores.

Sharding: 8 cores = 2 batches x 4 cores. Core c: batch b=c//4, j=c%4 owns
token supertile j (512 tokens) for all latent/q_b compute, heads [4j,4j+4)
for attention, and hid output rows [512j,512j+512) for o_proj.

Phases:
  P1   each core computes q/kv low-rank latents + RMSNorm + shared k_pe rope
       for ITS supertile; kv latents are AllGathered (GKV) early.
  QB   q_b + rope computed LOCALLY for ALL 16 heads on my 512 tokens (same
       PE cols as 4 heads x 2048 tokens), then one AllToAll exchanges
       [my tokens, all heads] -> [all tokens, my heads].  This removes the
       big q-latent AllGather from the critical path: the exchange happens
       after local compute instead of blocking it.
  P1b  per-head k_nope/V up-projections from the gathered kv latents.
  P2   attention per head: scores / exp / AV, 2-chunk [128,1024] PSUM
       groups; softmax denominators accumulated with DVE f16 adds + one
       ones-matmul per (head, q-supertile). exp shifted by -6. Causal mask
       multiplies restricted to the 128-wide diagonal triangle blocks
       (ranged accumulates skip the stale trimmed columns).
  P3   per q-supertile: LOCAL partial o_proj over my 4 heads producing
       out^T [2048 hid, 512 tok] f16 partials, then ReduceScatter(add)
       writes each core's 512 hid rows directly into the output tensor.
       The o_proj compute hides inside P2 of later supertiles and the tail
       is only the last ReduceScatter.

Layout: activations transposed [feature, token] on-chip. Host folds RMSNorm
gains + DeepSeek RoPE interleave permutation into the weights, and the
rotate-half SIGN into the sin table (rows 64:96 negated) so rotate-half is
two plain partition-shifted copies instead of a PE matmul. Matmul operands
f16, f32 PSUM. Input DMAs are spread across engine queues so the startup is
bandwidth- not queue-limited.
"""
import sys
import types

sys.path.insert(0, "/opt/trn_rl_repo")

import numpy as np

B, S, HID, NH = 2, 2048, 2048, 16
ROPE, NOPE, VDIM = 64, 64, 128
QHEAD, QLORA, KVLORA = 128, 682, 256
THETA = 128000.0
SCALE = 1.0 / float(np.sqrt(128.0))
EPS = 1e-6
NEGC = -6.0          # exp shift: exp(SCALE*s - 6), softmax-invariant
HPC = 4              # heads per core (attention)
NCORES = 8
QCH = [128, 128, 128, 128, 128, 42]   # qlora partition chunks
NST = 4              # 512-token supertiles per batch
STW = 512
GROUPS = [[0, 1, 2, 3], [4, 5, 6, 7]]

_PROGRAM = None


def _ensure_axon_hooks_shim():
    if "antenv.axon_hooks" in sys.modules:
        return
    try:
        from trn_agent_boot.trn_boot import _ntff_profile_via_ctypes
        hook = _ntff_profile_via_ctypes("/opt/axon/libaxon_pjrt.so")
    except Exception:
        hook = None
    m = types.ModuleType("antenv.axon_hooks")
    m.get_axon_ntff_profile_hook = lambda: hook
    m.set_axon_ntff_profile_hook = lambda h: None
    sys.modules["antenv.axon_hooks"] = m


def _build_program():
    import concourse.bass as bass  # noqa: F401
    import concourse.mybir as mybir
    import concourse.tile as tile
    from concourse import bacc

    f16 = mybir.dt.float16
    f32 = mybir.dt.float32
    AF = mybir.ActivationFunctionType

    nc = bacc.Bacc("TRN2", target_bir_lowering=False, debug=False,
                   num_devices=NCORES)
    for cv in (EPS, NEGC):
        t = nc.alloc_sbuf_tensor(f"const-{cv}", [128, 1], f32)
        nc.gpsimd.memset(t.ap(), cv)
        nc.const_aps.aps[(f32, cv)] = t.ap()
    nc.all_engine_barrier()

    def din(name, shape, dt=f16):
        return nc.dram_tensor(name, shape, dt, kind="ExternalInput").ap()

    xT = din("xT", [HID, STW])            # my supertile of x, transposed
    waq = din("waq", [HID, QLORA])        # q_a_w
    wakv = din("wakv", [HID, 320])        # kv_a_w cols: [ckv 256 | kpe-perm 64]
    wqb = din("wqb", [QLORA, HPC * 128])  # MY heads: [nope64|pe64-perm], ln folded
    wkn = din("wkn", [KVLORA, HPC * 64])  # my heads: knope cols, ln folded
    wv = din("wv", [KVLORA, HPC * 128])   # my heads: v cols, ln folded
    wo = din("wo", [HPC * VDIM, HID])     # MY heads' o_w rows x ALL hid cols
    cosT = din("cosT", [128, S])          # rows 0:64 = 1, rows 64:128 = cos
    sinT = din("sinT", [128, S])          # rows 0:64=0, 64:96=-sin[0:32], 96:128=sin[32:64]
    cosM = din("cosM", [64, STW])         # my supertile slice, rows 64:128 of cosT
    sinM = din("sinM", [64, STW])         # my supertile slice, rows 64:128 of sinT
    maskT = din("maskT", [128, 896])      # shifted causal window: m[k,c]=(k<=c-384)
    # out^T: rows = qs-major x (my 512 hid), cols = 512 toks of that qs
    out = nc.dram_tensor("out", [NST * STW, STW], f16, kind="ExternalOutput").ap()

    from contextlib import ExitStack
    with tile.TileContext(nc) as tc, ExitStack() as ctx:
        const = ctx.enter_context(tc.tile_pool(name="const", bufs=1))
        waqp = ctx.enter_context(tc.tile_pool(name="waqp", bufs=16))
        wakvp = ctx.enter_context(tc.tile_pool(name="wakvp", bufs=16))
        xtwo = ctx.enter_context(tc.tile_pool(name="xtwo", bufs=16))
        qrawp = ctx.enter_context(tc.tile_pool(name="qrawp", bufs=6))
        lrsp = ctx.enter_context(tc.tile_pool(name="lrsp", bufs=2))
        wop = ctx.enter_context(tc.tile_pool(name="wop", bufs=HPC))
        latkp = ctx.enter_context(tc.tile_pool(name="latkp", bufs=7))
        rawp = ctx.enter_context(tc.tile_pool(name="rawp", bufs=1))
        sqp = ctx.enter_context(tc.tile_pool(name="sqp", bufs=1))
        smallp = ctx.enter_context(tc.tile_pool(name="smallp", bufs=2))
        bcp = ctx.enter_context(tc.tile_pool(name="bcp", bufs=2))
        scr1 = ctx.enter_context(tc.tile_pool(name="scr1", bufs=1))
        persist = ctx.enter_context(tc.tile_pool(name="persist", bufs=HPC))
        ptp = ctx.enter_context(tc.tile_pool(name="ptp", bufs=2))
        accp = ctx.enter_context(tc.tile_pool(name="accp", bufs=4))
        aoutp = ctx.enter_context(tc.tile_pool(name="aoutp", bufs=8))
        dramp = ctx.enter_context(tc.tile_pool(name="dram", bufs=1, space="DRAM"))
        # PSUM: 8 banks = big 2x[128,1024] (4) + out 3x[128,512] (3) + misc 1
        ps_big = ctx.enter_context(tc.tile_pool(name="ps_big", bufs=2, space="PSUM"))
        ps_out = ctx.enter_context(tc.tile_pool(name="ps_out", bufs=3, space="PSUM"))
        ps_misc = ctx.enter_context(tc.tile_pool(name="ps_misc", bufs=1, space="PSUM"))

        # ---- constants / weights into SBUF, spread across engine queues ----
        sb_waq = [waqp.tile([128, QLORA], f16, tag="waq", name=f"waq{hc}")
                  for hc in range(16)]
        sb_wakv = [wakvp.tile([128, 320], f16, tag="wakv", name=f"wakv{hc}")
                   for hc in range(16)]
        W = HPC * 128
        sb_wqb = const.tile([128, 6 * W], f16, tag="wqb")
        sb_wkn = const.tile([128, 2 * HPC * 64], f16, tag="wkn")
        sb_wv = const.tile([128, 2 * HPC * 128], f16, tag="wv")
        sb_cos = const.tile([128, S], f16, tag="cos")
        sb_sin = const.tile([128, S], f16, tag="sin")
        sb_cosM = const.tile([64, STW], f16, tag="cosM")
        sb_sinM = const.tile([64, STW], f16, tag="sinM")
        sb_mask = const.tile([128, 896], f16, tag="mask")
        sb_ones = const.tile([128, 1], f16, tag="ones")
        sb_onesr = const.tile([1, 128], f16, tag="onesr")

        qoff = [0, 128, 256, 384, 512, 640]
        xt = [xtwo.tile([128, STW], f16, tag="xt", name=f"xt{hc}")
              for hc in range(16)]
        # sync queue: xt (needed first, for kv latents)
        for hc in range(16):
            nc.sync.dma_start(out=xt[hc][:], in_=xT[hc * 128:(hc + 1) * 128, :])
        # scalar queue: wakv (needed first), then waq (q latents)
        for hc in range(16):
            nc.scalar.dma_start(out=sb_wakv[hc][:],
                                in_=wakv[hc * 128:(hc + 1) * 128, :])
        for hc in range(16):
            nc.scalar.dma_start(out=sb_waq[hc][:],
                                in_=waq[hc * 128:(hc + 1) * 128, :])
        # gpsimd queue: rope tables, mask, then wqb
        nc.gpsimd.dma_start(out=sb_cosM[:], in_=cosM[:])
        nc.gpsimd.dma_start(out=sb_sinM[:], in_=sinM[:])
        nc.gpsimd.dma_start(out=sb_cos[:], in_=cosT[:])
        nc.gpsimd.dma_start(out=sb_sin[:], in_=sinT[:])
        nc.gpsimd.dma_start(out=sb_mask[:], in_=maskT[:])
        for c in range(6):
            nc.gpsimd.dma_start(out=sb_wqb[:QCH[c], c * W:(c + 1) * W],
                                in_=wqb[qoff[c]:qoff[c] + QCH[c], :])
        for c in range(2):
            nc.gpsimd.dma_start(out=sb_wkn[:, c * HPC * 64:(c + 1) * HPC * 64],
                                in_=wkn[c * 128:(c + 1) * 128, :])
            nc.gpsimd.dma_start(out=sb_wv[:, c * HPC * 128:(c + 1) * HPC * 128],
                                in_=wv[c * 128:(c + 1) * 128, :])
        nc.vector.memset(sb_ones[:], 1.0)
        nc.vector.memset(sb_onesr[:], 1.0)
        # wo loads on sync queue behind xt
        sb_wo = []
        for h in range(HPC):
            t = wop.tile([128, HID], f16, tag="wo", name=f"wo{h}")
            nc.sync.dma_start(out=t[:], in_=wo[h * 128:(h + 1) * 128, :])
            sb_wo.append(t)

        # persistent per-head tensors (my 4 heads x all tokens)
        qfT = [persist.tile([128, S], f16, tag="qf", name=f"qfT{h}") for h in range(HPC)]
        kfT = [persist.tile([128, S], f16, tag="kf", name=f"kfT{h}") for h in range(HPC)]
        VT = [persist.tile([128, 16 * VDIM], f16, tag="vh", name=f"VT{h}") for h in range(HPC)]

        # DRAM bounce buffers
        bgk_in = dramp.tile([128, 1536], f16, name="bgk_in", tag="bgk_in")
        bgk_out = dramp.tile([4 * 128, 1536], f16, name="bgk_out", tag="bgk_out")
        bgq_in = dramp.tile([128, 3072], f16, name="bgq_in", tag="bgq_in")
        bgq_out = dramp.tile([4 * 128, 3072], f16, name="bgq_out", tag="bgq_out")

        # ================= P1: latents for MY supertile =================
        sums = ps_out.tile([64, STW], f32, tag="out", name="sums")  # row0 q, row32 k

        # kv latents: ckv (2 chunk halves) + kpe (64 rows), then GKV gather
        psk = ps_big.tile([128, 1024], f32, tag="big", name="klat")
        for half in range(2):
            for hc in range(16):
                nc.tensor.matmul(
                    psk[:, half * STW:(half + 1) * STW],
                    sb_wakv[hc][:, half * 128:(half + 1) * 128],
                    xt[hc][:],
                    start=(hc == 0), stop=(hc == 15))
        kraw = rawp.tile([128, 1024], f16, tag="rawk")
        nc.vector.tensor_copy(kraw[:], psk[:])
        sqk = sqp.tile([128, 1024], f16, tag="sq")
        nc.scalar.activation(sqk[:], psk[:], AF.Square)
        for half in range(2):
            nc.tensor.matmul(sums[32:33, :], sb_ones[:, :],
                             sqk[:, half * STW:(half + 1) * STW],
                             start=(half == 0), stop=(half == 1))

        psp = ps_misc.tile([128, STW], f32, tag="misc", name="kpelat")
        for hc in range(16):
            nc.tensor.matmul(psp[0:64, :], sb_wakv[hc][:, 256:320], xt[hc][:],
                             start=(hc == 0), stop=(hc == 15))
        kperaw = rawp.tile([64, STW], f16, tag="kpe")
        nc.vector.tensor_copy(kperaw[:], psp[0:64, :])

        # rstd for k, normalize kraw
        stdk = smallp.tile([1, STW], f32, tag="stdk", bufs=1)
        nc.scalar.activation(stdk[:], sums[32:33, :], AF.Sqrt,
                             bias=EPS, scale=1.0 / KVLORA)
        rstdkf = smallp.tile([1, STW], f32, tag="rstdkf", bufs=1)
        nc.vector.reciprocal_approx_fast(out=rstdkf[:], in_=stdk[:])
        rstdk = smallp.tile([1, STW], f16, tag="rstdk", bufs=1)
        nc.vector.tensor_copy(rstdk[:], rstdkf[:])
        bck = ps_misc.tile([128, STW], f32, tag="misc", name="bck")
        nc.tensor.matmul(bck[:], sb_onesr[:], rstdk[:1, :], start=True, stop=True)
        bcks = bcp.tile([128, STW], f16, tag="bc", name="bcks")
        nc.vector.tensor_copy(bcks[:], bck[:])
        nc.vector.tensor_mul(kraw[:, 0:STW], kraw[:, 0:STW], bcks[:])
        nc.vector.tensor_mul(kraw[:, STW:1024], kraw[:, STW:1024], bcks[:])

        # shared k_pe rope on [64, 512]: rotate-half = 2 plain swapped copies
        # (sign folded into sinM rows 64:96); tables rows 64:128.
        rotk = scr1.tile([64, STW], f16, tag="rotk")
        nc.scalar.copy(out=rotk[0:32, :], in_=kperaw[32:64, :])
        nc.scalar.copy(out=rotk[32:64, :], in_=kperaw[0:32, :])
        t1k = scr1.tile([64, STW], f16, tag="t1k")
        nc.vector.tensor_mul(t1k[:], rotk[:], sb_sinM[:])
        t2k = scr1.tile([64, STW], f16, tag="t2k")
        nc.vector.tensor_mul(t2k[:], kperaw[:], sb_cosM[:])
        kpero = kperaw
        nc.vector.tensor_add(kpero[:], t1k[:], t2k[:])

        nc.gpsimd.dma_start(bgk_in[:, 0:1024], kraw[:])
        nc.gpsimd.dma_start(bgk_in[64:128, 1024:1536], kpero[:])
        nc.gpsimd.collective_compute(
            "AllGather", mybir.AluOpType.bypass, replica_groups=GROUPS,
            ins=[bgk_in.opt()], outs=[bgk_out.opt()])

        # q latents: 3 chunk-pairs, normalized pre-gather
        qraw = []
        for p in range(3):
            ps = ps_big.tile([128, 1024], f32, tag="big", name=f"qlat{p}")
            for half in range(2):
                c = 2 * p + half
                for hc in range(16):
                    nc.tensor.matmul(
                        ps[:QCH[c], half * STW:(half + 1) * STW],
                        sb_waq[hc][:, qoff[c]:qoff[c] + QCH[c]],
                        xt[hc][:],
                        start=(hc == 0), stop=(hc == 15))
            raw = qrawp.tile([128, 1024], f16, tag="rawq", name=f"rawq{p}",
                             bufs=6)
            nc.vector.tensor_copy(raw[:], ps[:])
            sq = sqp.tile([128, 1024], f16, tag="sq")
            nc.scalar.activation(sq[:], ps[:], AF.Square)
            for half in range(2):
                c = 2 * p + half
                nc.tensor.matmul(sums[0:1, :], sb_ones[:QCH[c], :],
                                 sq[:QCH[c], half * STW:(half + 1) * STW],
                                 start=(c == 0), stop=(c == 5))
            qraw.append(raw)

        # rstd for q, applied pre-gather (payload stays 512-col aligned)
        stdq = smallp.tile([1, STW], f32, tag="stdq", bufs=1)
        nc.scalar.activation(stdq[:], sums[0:1, :], AF.Sqrt,
                             bias=EPS, scale=1.0 / QLORA)
        rstdqf = smallp.tile([1, STW], f32, tag="rstdqf", bufs=1)
        nc.vector.reciprocal_approx_fast(out=rstdqf[:], in_=stdq[:])
        rstdq = smallp.tile([1, STW], f16, tag="rstdq", bufs=1)
        nc.vector.tensor_copy(rstdq[:], rstdqf[:])
        bcq = ps_misc.tile([128, STW], f32, tag="misc", name="bcq")
        nc.tensor.matmul(bcq[:], sb_onesr[:], rstdq[:1, :], start=True, stop=True)
        bcqs = bcp.tile([128, STW], f16, tag="bc", name="bcqs")
        nc.scalar.copy(out=bcqs[:], in_=bcq[:])
        for p in range(3):
            for half in range(2):
                nc.vector.tensor_mul(
                    qraw[p][:, half * STW:(half + 1) * STW],
                    qraw[p][:, half * STW:(half + 1) * STW], bcqs[:])
            nc.gpsimd.dma_start(bgq_in[:, p * 1024:(p + 1) * 1024], qraw[p][:])
        nc.gpsimd.collective_compute(
            "AllGather", mybir.AluOpType.bypass, replica_groups=GROUPS,
            ins=[bgq_in.opt()], outs=[bgq_out.opt()])

        # q_b + rope for one gathered supertile
        def qb_block(st):
            cols = slice(st * STW, (st + 1) * STW)
            gq = []
            for p in range(3):
                t = qrawp.tile([128, 1024], f16, tag="rawq",
                               name=f"gq{st}_{p}", bufs=6)
                nc.sync.dma_start(
                    t[:], bgq_out[st * 128:(st + 1) * 128,
                                  p * 1024:(p + 1) * 1024])
                gq.append(t)
            for pr in range(2):
                qra = scr1.tile([128, 2 * STW], f16, tag="qra",
                                name=f"qra{st}_{pr}", bufs=1)
                for i in range(2):
                    h = 2 * pr + i
                    psq = ps_out.tile([128, STW], f32, tag="out",
                                      name=f"psq{st}_{h}")
                    for c in range(6):
                        nc.tensor.matmul(
                            psq[:],
                            sb_wqb[:QCH[c], c * W + h * 128:c * W + (h + 1) * 128],
                            gq[c // 2][:QCH[c], (c % 2) * STW:(c % 2 + 1) * STW],
                            start=(c == 0), stop=(c == 5))
                    # nope rows go straight to qfT (no rstd mul needed)
                    nc.vector.tensor_copy(qfT[h][0:64, cols], psq[0:64, :])
                    nc.vector.tensor_copy(qra[64:128, i * STW:(i + 1) * STW],
                                          psq[64:128, :])
                # rope: rows 0:64 = qra*cosq (nope, rstd); 64:128 full rope,
                # rotate-half = 2 plain swapped copies (sign in sin table)
                for i in range(2):
                    h = 2 * pr + i
                    hs = slice(i * STW, (i + 1) * STW)
                    rotc = scr1.tile([128, STW], f16, tag="rotc",
                                     name=f"rotc{st}_{pr}_{i}", bufs=2)
                    nc.scalar.copy(out=rotc[64:96, :], in_=qra[96:128, hs])
                    nc.scalar.copy(out=rotc[96:128, :], in_=qra[64:96, hs])
                    nc.vector.tensor_mul(rotc[64:128, :], rotc[64:128, :],
                                         sb_sin[64:128, cols])
                    nc.vector.tensor_mul(qra[64:128, hs], qra[64:128, hs],
                                         sb_cos[64:128, cols])
                    nc.vector.tensor_add(qfT[h][64:128, cols],
                                         rotc[64:128, :], qra[64:128, hs])

        # ================= P1b: kn/V over all supertiles (needs GKV) =======
        for st in range(NST):
            cols = slice(st * STW, (st + 1) * STW)
            gk = [latkp.tile([128, STW], f16, tag="latk", name=f"gk{st}_{c}")
                  for c in range(2)]
            for c in range(2):
                nc.sync.dma_start(
                    gk[c][:],
                    bgk_out[st * 128:(st + 1) * 128, c * STW:(c + 1) * STW])
            # shared k_pe straight into each head's kfT rows 64:128
            for h in range(HPC):
                eng = nc.scalar if h % 2 == 0 else nc.gpsimd
                eng.dma_start(
                    out=kfT[h][64:128, cols],
                    in_=bgk_out[st * 128 + 64:(st + 1) * 128, 1024:1536])
            for hp in range(2):
                pskn = ps_out.tile([128, STW], f32, tag="out", name=f"kn{st}_{hp}")
                for c in range(2):
                    nc.tensor.matmul(
                        pskn[:],
                        sb_wkn[:, c * HPC * 64 + hp * 128:c * HPC * 64 + (hp + 1) * 128],
                        gk[c][:],
                        start=(c == 0), stop=(c == 1))
                nc.scalar.copy(out=kfT[2 * hp][0:64, cols], in_=pskn[0:64, :])
                nc.scalar.copy(out=kfT[2 * hp + 1][0:64, cols], in_=pskn[64:128, :])
            for h in range(HPC):
                psv = ps_out.tile([128, STW], f32, tag="out", name=f"psv{st}_{h}")
                for tcn in range(4):
                    for c in range(2):
                        nc.tensor.matmul(
                            psv[:, tcn * VDIM:(tcn + 1) * VDIM],
                            gk[c][:, tcn * 128:(tcn + 1) * 128],
                            sb_wv[:, c * HPC * 128 + h * 128:c * HPC * 128 + (h + 1) * 128],
                            start=(c == 0), stop=(c == 1))
                nc.vector.tensor_copy(VT[h][:, st * STW:(st + 1) * STW], psv[:])

        # ================= P2 / P3, per q-supertile ========================
        # pre-zero pt slots: qs==0 diagonal chunks use full-width mask muls
        # that must see finite values in the stale trimmed columns
        for z in range(2):
            ptz = ptp.tile([128, 1024], f16, tag="pt", name=f"ptz{z}")
            nc.vector.memset(ptz[:], 0.0)

        def norm(qs, h, ssums, accs, aouts):
            bca = ps_misc.tile([128, STW], f32, tag="misc", name=f"bca{qs}_{h}")
            nc.tensor.matmul(bca[:], sb_onesr[:], ssums[h][:1, :],
                             start=True, stop=True)
            bcas = bcp.tile([128, STW], f16, tag="bc", name=f"bcas{qs}_{h}")
            nc.scalar.copy(out=bcas[:], in_=bca[:])
            ao = aoutp.tile([128, STW], f16, tag="aout", name=f"ao{qs}_{h}")
            nc.vector.tensor_mul(ao[:], accs[h][:], bcas[:])
            aouts.append(ao)

        def p2_block(qs):
            nkc = 4 * qs + 4
            ng = nkc // 2
            accs, ssums, aouts = [], [], []
            kcs = (list(range(4 * qs, nkc)) + list(range(0, 4 * qs))
                   if qs > 0 else list(range(nkc)))
            for h in range(HPC):
                outT = ps_out.tile([128, STW], f32, tag="out", name=f"oT{qs}_{h}")
                # two partial prob-sum accumulators: even chunks chain on
                # DVE (accA), odd chunks on GpSimd (accB); the combine is
                # folded into the ssum PSUM accumulation
                accA = accp.tile([128, STW], f16, tag="acc", name=f"accA{qs}_{h}")
                accB = accp.tile([128, STW], f16, tag="acc", name=f"accB{qs}_{h}")

                stps, pts = {}, {}
                def sc(g):
                    stp = ps_big.tile([128, 1024], f32, tag="big",
                                      name=f"sc{qs}_{h}_{g}")
                    trim = [0, 0]
                    for half in range(2):
                        kc = kcs[2 * g + half]
                        j = kc - 4 * qs
                        tr = 128 * j if j > 0 else 0
                        trim[half] = tr
                        nc.tensor.matmul(
                            stp[:, half * STW + tr:(half + 1) * STW],
                            kfT[h][:, kc * 128:(kc + 1) * 128],
                            qfT[h][:, qs * STW + tr:(qs + 1) * STW],
                            start=True, stop=True)
                    pt = ptp.tile([128, 1024], f16, tag="pt")
                    if trim[0] == 0 and trim[1] == 0:
                        nc.scalar.activation(pt[:], stp[:], AF.Exp,
                                             bias=NEGC, scale=SCALE)
                    else:
                        for half in range(2):
                            tr = trim[half]
                            nc.scalar.activation(
                                pt[:, half * STW + tr:(half + 1) * STW],
                                stp[:, half * STW + tr:(half + 1) * STW],
                                AF.Exp, bias=NEGC, scale=SCALE)
                    for half in range(2):
                        kc = kcs[2 * g + half]
                        j = kc - 4 * qs
                        if j >= 0:
                            tr = 128 * j
                            if qs == 0 and j > 0:
                                # full-width mask (zeroes stale cols too)
                                ph = pt[:, half * STW:(half + 1) * STW]
                                nc.vector.tensor_mul(
                                    ph, ph,
                                    sb_mask[:, 384 - tr:896 - tr])
                            else:
                                # triangle block only
                                blk = slice(half * STW + tr,
                                            half * STW + tr + 128)
                                nc.vector.tensor_mul(
                                    pt[:, blk], pt[:, blk],
                                    sb_mask[:, 384:512])
                    pts[g] = pt

                def av(g, first):
                    pt = pts.pop(g)
                    for half in range(2):
                        idx = 2 * g + half
                        kc = kcs[idx]
                        j = kc - 4 * qs
                        tr = 128 * j if (j > 0 and qs > 0) else 0
                        ph = pt[:, half * STW + tr:(half + 1) * STW]
                        nc.tensor.matmul(outT[:, tr:STW],
                                         VT[h][:, kc * VDIM:(kc + 1) * VDIM],
                                         ph,
                                         start=(idx == 0), stop=(idx == nkc - 1))
                        ph2 = pt[:, half * STW + tr:(half + 1) * STW]
                        if idx == 0:
                            first[0] = pt[:, half * STW:(half + 1) * STW]
                        elif idx == 1:
                            if tr > 0:
                                nc.gpsimd.memset(accB[:, 0:tr], 0.0)
                                nc.gpsimd.tensor_copy(accB[:, tr:STW], ph2)
                            else:
                                nc.gpsimd.tensor_copy(
                                    accB[:], pt[:, half * STW:(half + 1) * STW])
                        elif idx == 2:
                            if tr > 0:
                                nc.vector.tensor_copy(accA[:, 0:tr],
                                                      first[0][:, 0:tr])
                            nc.vector.tensor_add(
                                accA[:, tr:STW], first[0][:, tr:STW], ph2)
                        elif idx % 2 == 0:
                            nc.vector.tensor_add(
                                accA[:, tr:STW], accA[:, tr:STW], ph2)
                        else:
                            nc.gpsimd.tensor_add(
                                accB[:, tr:STW], accB[:, tr:STW], ph2)

                first = [None]
                sc(0)
                if h >= 1:
                    norm(qs, h - 1, ssums, accs, aouts)
                for g in range(ng):
                    if g + 1 < ng:
                        sc(g + 1)
                    av(g, first)
                ssum = ps_misc.tile([1, STW], f32, tag="misc", name=f"ss{qs}_{h}")
                nc.tensor.matmul(ssum[:], sb_ones[:, :], accA[:],
                                 start=True, stop=False)
                nc.tensor.matmul(ssum[:], sb_ones[:, :], accB[:],
                                 start=False, stop=True)
                rsf = smallp.tile([1, STW], f32, tag="rsf", bufs=2)
                nc.vector.reciprocal_approx_fast(out=rsf[:], in_=ssum[:])
                rs = smallp.tile([1, STW], f16, tag="rs", bufs=4)
                nc.vector.tensor_copy(rs[:], rsf[:])
                ssums.append(rs)
                accs.append(outT)
            norm(qs, HPC - 1, ssums, accs, aouts)
            return aouts

        def p3_block(qs, aouts):
            # local partial o_proj: out^T [2048 hid, 512 tok] over my 4 heads
            rs_in = dramp.tile([NH * 128, STW], f16, tag="rs_in",
                               name=f"rsin{qs}", bufs=2)
            for hcn in range(16):
                pso = ps_out.tile([128, STW], f32, tag="out", name=f"pso{qs}_{hcn}")
                for h in range(HPC):
                    nc.tensor.matmul(
                        pso[:],
                        sb_wo[h][:, hcn * 128:(hcn + 1) * 128],
                        aouts[h][:],
                        start=(h == 0), stop=(h == HPC - 1))
                ob = waqp.tile([128, STW], f16, tag="waq", name=f"ob{qs}_{hcn}")
                if hcn % 2 == 0:
                    nc.scalar.copy(out=ob[:], in_=pso[:])
                else:
                    nc.vector.tensor_copy(ob[:], pso[:])
                deng = nc.sync if hcn % 2 == 0 else nc.gpsimd
                deng.dma_start(rs_in[hcn * 128:(hcn + 1) * 128, :], ob[:])
            rs_out = dramp.tile([STW, STW], f16, tag="rs_out",
                                name=f"rsout{qs}", bufs=2)
            nc.gpsimd.collective_compute(
                "ReduceScatter", mybir.AluOpType.add, replica_groups=GROUPS,
                ins=[rs_in.opt()], outs=[rs_out.opt()])
            nc.sync.dma_start(out[qs * STW:(qs + 1) * STW, :], rs_out[:])

        for st in range(NST):
            qb_block(st)
        for qs in range(NST):
            aouts = p2_block(qs)
            p3_block(qs, aouts)

    nc.compile()
    return nc


def _host_prep(inputs):
    f16 = np.float16
    x = np.asarray(inputs["x"], np.float32)
    q_a_w = np.asarray(inputs["q_a_w"], np.float32)
    q_a_ln = np.asarray(inputs["q_a_ln_w"], np.float32)
    q_b_w = np.asarray(inputs["q_b_w"], np.float32)
    kv_a_w = np.asarray(inputs["kv_a_w"], np.float32)
    kv_a_ln = np.asarray(inputs["kv_a_ln_w"], np.float32)
    kv_b_w = np.asarray(inputs["kv_b_w"], np.float32)
    o_w = np.asarray(inputs["o_w"], np.float32)

    perm = np.concatenate([np.arange(0, ROPE, 2), np.arange(1, ROPE, 2)])
    q_b_f = q_b_w * q_a_ln[:, None]
    kv_b_f = kv_b_w * kv_a_ln[:, None]

    # kv_a: [ckv 256 | kpe perm 64]
    wakv = np.concatenate(
        [kv_a_w[:, :KVLORA], kv_a_w[:, KVLORA:][:, perm]], axis=1).astype(f16)
    waq = q_a_w.astype(f16)

    # rope tables (transposed [dim, pos]); rotate-half sign folded into sin:
    # rows 64:96 = -sin[0:32], rows 96:128 = +sin[32:64]
    inv = 1.0 / (THETA ** (np.arange(0, ROPE, 2, dtype=np.float64) / ROPE))
    freqs = np.outer(np.arange(S, dtype=np.float64), inv)      # [S, 32]
    cos64 = np.concatenate([np.cos(freqs), np.cos(freqs)], -1).T  # [64, S]
    sin64 = np.concatenate([np.sin(freqs), np.sin(freqs)], -1).T
    cosT = np.concatenate([np.ones((64, S)), cos64], 0).astype(f16)
    sinT = np.concatenate([np.zeros((64, S)), -sin64[0:32], sin64[32:64]],
                          0).astype(f16)

    # shifted causal window: maskT[k, c] = k <= c - 384; cols 384:512 are the
    # plain [128,128] triangle k <= q'
    k_i = np.arange(128)[:, None]
    c_i = np.arange(896)[None, :]
    maskT = (k_i <= c_i - 384).astype(f16)

    in_maps = []
    for core in range(NCORES):
        b = core // 4
        j = core % 4
        heads = [HPC * j + i for i in range(HPC)]
        wqb = np.concatenate(
            [np.concatenate(
                [q_b_f[:, h * QHEAD:h * QHEAD + NOPE],
                 q_b_f[:, h * QHEAD + NOPE:(h + 1) * QHEAD][:, perm]], 1)
             for h in heads], axis=1).astype(f16)
        wkn = np.concatenate(
            [kv_b_f[:, h * (NOPE + VDIM):h * (NOPE + VDIM) + NOPE]
             for h in heads], axis=1).astype(f16)
        wv = np.concatenate(
            [kv_b_f[:, h * (NOPE + VDIM) + NOPE:(h + 1) * (NOPE + VDIM)]
             for h in heads], axis=1).astype(f16)
        wo = np.concatenate(
            [o_w[h * VDIM:(h + 1) * VDIM, :] for h in heads], axis=0).astype(f16)
        scols = slice(j * STW, (j + 1) * STW)
        in_maps.append({
            "xT": np.ascontiguousarray(x[b].T[:, scols]).astype(f16),
            "waq": waq, "wakv": wakv, "wqb": wqb, "wkn": wkn, "wv": wv,
            "wo": wo, "cosT": cosT, "sinT": sinT,
            "cosM": np.ascontiguousarray(cosT[64:128, scols]),
            "sinM": np.ascontiguousarray(sinT[64:128, scols]),
            "maskT": maskT,
        })
    return in_maps


def kernel(**inputs):
    global _PROGRAM
    _ensure_axon_hooks_shim()
    from concourse.bass_utils import run_bass_kernel_spmd

    if _PROGRAM is None:
        _PROGRAM = _build_program()
    in_maps = _host_prep(inputs)
    res = run_bass_kernel_spmd(_PROGRAM, in_maps, list(range(NCORES)))
    out = np.zeros((B, S, HID), np.float32)
    for core in range(NCORES):
        b, j = core // 4, core % 4
        r = res.results[core]["out"].astype(np.float32)   # [4*512 hid-qs, 512]
        for qs in range(NST):
            out[b][qs * STW:(qs + 1) * STW, j * STW:(j + 1) * STW] = \
                r[qs * STW:(qs + 1) * STW, :].T
    return out


# revision 45
# speedup vs baseline: 1.0357x; 1.0268x over previous
"""DeepSeek MLA head — Trainium2 Bass kernel, 8 NeuronCores.

Sharding: 8 cores = 2 batches x 4 cores. Core c: batch b=c//4, j=c%4 owns
token supertile j (512 tokens) for all latent/q_b compute, heads [4j,4j+4)
for attention, and hid output rows [512j,512j+512) for o_proj.

Phases:
  P1   each core computes q/kv low-rank latents + RMSNorm + shared k_pe rope
       for ITS supertile; kv latents are AllGathered (GKV) early.
  QB   q_b + rope computed LOCALLY for ALL 16 heads on my 512 tokens (same
       PE cols as 4 heads x 2048 tokens), then one AllToAll exchanges
       [my tokens, all heads] -> [all tokens, my heads].  This removes the
       big q-latent AllGather from the critical path: the exchange happens
       after local compute instead of blocking it.
  P1b  per-head k_nope/V up-projections from the gathered kv latents.
  P2   attention per head: scores / exp / AV, 2-chunk [128,1024] PSUM
       groups; softmax denominators accumulated with DVE f16 adds + one
       ones-matmul per (head, q-supertile). exp shifted by -6. Causal mask
       multiplies restricted to the 128-wide diagonal triangle blocks
       (ranged accumulates skip the stale trimmed columns).
  P3   per q-supertile: LOCAL partial o_proj over my 4 heads producing
       out^T [2048 hid, 512 tok] f16 partials, then ReduceScatter(add)
       writes each core's 512 hid rows directly into the output tensor.
       The o_proj compute hides inside P2 of later supertiles and the tail
       is only the last ReduceScatter.

Layout: activations transposed [feature, token] on-chip. Host folds RMSNorm
gains + DeepSeek RoPE interleave permutation into the weights, and the
rotate-half SIGN into the sin table (rows 64:96 negated) so rotate-half is
two plain partition-shifted copies instead of a PE matmul. Matmul operands
f16, f32 PSUM. Input DMAs are spread across engine queues so the startup is
bandwidth- not queue-limited.
"""
import sys
import types

sys.path.insert(0, "/opt/trn_rl_repo")

import numpy as np

B, S, HID, NH = 2, 2048, 2048, 16
ROPE, NOPE, VDIM = 64, 64, 128
QHEAD, QLORA, KVLORA = 128, 682, 256
THETA = 128000.0
SCALE = 1.0 / float(np.sqrt(128.0))
EPS = 1e-6
NEGC = -6.0          # exp shift: exp(SCALE*s - 6), softmax-invariant
HPC = 4              # heads per core (attention)
NCORES = 8
QCH = [128, 128, 128, 128, 128, 42]   # qlora partition chunks
NST = 4              # 512-token supertiles per batch
STW = 512
GROUPS = [[0, 1, 2, 3], [4, 5, 6, 7]]

_PROGRAM = None


def _ensure_axon_hooks_shim():
    if "antenv.axon_hooks" in sys.modules:
        return
    try:
        from trn_agent_boot.trn_boot import _ntff_profile_via_ctypes
        hook = _ntff_profile_via_ctypes("/opt/axon/libaxon_pjrt.so")
    except Exception:
        hook = None
    m = types.ModuleType("antenv.axon_hooks")
    m.get_axon_ntff_profile_hook = lambda: hook
    m.set_axon_ntff_profile_hook = lambda h: None
    sys.modules["antenv.axon_hooks"] = m


def _build_program():
    import concourse.bass as bass  # noqa: F401
    import concourse.mybir as mybir
    import concourse.tile as tile
    from concourse import bacc

    f16 = mybir.dt.float16
    f32 = mybir.dt.float32
    AF = mybir.ActivationFunctionType

    nc = bacc.Bacc("TRN2", target_bir_lowering=False, debug=False,
                   num_devices=NCORES)
    for cv in (EPS, NEGC):
        t = nc.alloc_sbuf_tensor(f"const-{cv}", [128, 1], f32)
        nc.gpsimd.memset(t.ap(), cv)
        nc.const_aps.aps[(f32, cv)] = t.ap()
    nc.all_engine_barrier()

    def din(name, shape, dt=f16):
        return nc.dram_tensor(name, shape, dt, kind="ExternalInput").ap()

    xT = din("xT", [HID, STW])            # my supertile of x, transposed
    waq = din("waq", [HID, QLORA])        # q_a_w
    wakv = din("wakv", [HID, 320])        # kv_a_w cols: [ckv 256 | kpe-perm 64]
    wqb = din("wqb", [QLORA, HPC * 128])  # MY heads: [nope64|pe64-perm], ln folded
    wkn = din("wkn", [KVLORA, HPC * 64])  # my heads: knope cols, ln folded
    wv = din("wv", [KVLORA, HPC * 128])   # my heads: v cols, ln folded
    wo = din("wo", [HPC * VDIM, HID])     # MY heads' o_w rows x ALL hid cols
    cosT = din("cosT", [128, S])          # rows 0:64 = 1, rows 64:128 = cos
    sinT = din("sinT", [128, S])          # rows 0:64=0, 64:96=-sin[0:32], 96:128=sin[32:64]
    cosM = din("cosM", [64, STW])         # my supertile slice, rows 64:128 of cosT
    sinM = din("sinM", [64, STW])         # my supertile slice, rows 64:128 of sinT
    maskT = din("maskT", [128, 896])      # shifted causal window: m[k,c]=(k<=c-384)
    # out^T: rows = qs-major x (my 512 hid), cols = 512 toks of that qs
    out = nc.dram_tensor("out", [NST * STW, STW], f16, kind="ExternalOutput").ap()

    from contextlib import ExitStack
    with tile.TileContext(nc) as tc, ExitStack() as ctx:
        const = ctx.enter_context(tc.tile_pool(name="const", bufs=1))
        waqp = ctx.enter_context(tc.tile_pool(name="waqp", bufs=16))
        wakvp = ctx.enter_context(tc.tile_pool(name="wakvp", bufs=16))
        xtwo = ctx.enter_context(tc.tile_pool(name="xtwo", bufs=16))
        qrawp = ctx.enter_context(tc.tile_pool(name="qrawp", bufs=6))
        lrsp = ctx.enter_context(tc.tile_pool(name="lrsp", bufs=2))
        wop = ctx.enter_context(tc.tile_pool(name="wop", bufs=HPC))
        latkp = ctx.enter_context(tc.tile_pool(name="latkp", bufs=7))
        rawp = ctx.enter_context(tc.tile_pool(name="rawp", bufs=1))
        sqp = ctx.enter_context(tc.tile_pool(name="sqp", bufs=1))
        smallp = ctx.enter_context(tc.tile_pool(name="smallp", bufs=2))
        bcp = ctx.enter_context(tc.tile_pool(name="bcp", bufs=2))
        scr1 = ctx.enter_context(tc.tile_pool(name="scr1", bufs=1))
        persist = ctx.enter_context(tc.tile_pool(name="persist", bufs=HPC))
        ptp = ctx.enter_context(tc.tile_pool(name="ptp", bufs=2))
        accp = ctx.enter_context(tc.tile_pool(name="accp", bufs=4))
        aoutp = ctx.enter_context(tc.tile_pool(name="aoutp", bufs=8))
        dramp = ctx.enter_context(tc.tile_pool(name="dram", bufs=1, space="DRAM"))
        # PSUM: 8 banks = big 2x[128,1024] (4) + out 3x[128,512] (3) + misc 1
        ps_big = ctx.enter_context(tc.tile_pool(name="ps_big", bufs=2, space="PSUM"))
        ps_out = ctx.enter_context(tc.tile_pool(name="ps_out", bufs=3, space="PSUM"))
        ps_misc = ctx.enter_context(tc.tile_pool(name="ps_misc", bufs=1, space="PSUM"))

        # ---- warmup collective: absorbs CC stream spin-up + peer skew
        # while the weight DMAs stream in
        warm_s = const.tile([1, 8], f16, tag="warm")
        nc.vector.memset(warm_s[:], 0.0)
        warm_in = dramp.tile([1, 8], f16, name="warm_in", tag="warm_in")
        warm_out = dramp.tile([4, 8], f16, name="warm_out", tag="warm_out")
        nc.gpsimd.dma_start(warm_in[:], warm_s[:])
        nc.gpsimd.collective_compute(
            "AllGather", mybir.AluOpType.bypass, replica_groups=GROUPS,
            ins=[warm_in.opt()], outs=[warm_out.opt()])

        # ---- constants / weights into SBUF, spread across engine queues ----
        sb_waq = [waqp.tile([128, QLORA], f16, tag="waq", name=f"waq{hc}")
                  for hc in range(16)]
        sb_wakv = [wakvp.tile([128, 320], f16, tag="wakv", name=f"wakv{hc}")
                   for hc in range(16)]
        W = HPC * 128
        sb_wqb = const.tile([128, 6 * W], f16, tag="wqb")
        sb_wkn = const.tile([128, 2 * HPC * 64], f16, tag="wkn")
        sb_wv = const.tile([128, 2 * HPC * 128], f16, tag="wv")
        sb_cos = const.tile([128, S], f16, tag="cos")
        sb_sin = const.tile([128, S], f16, tag="sin")
        sb_cosM = const.tile([64, STW], f16, tag="cosM")
        sb_sinM = const.tile([64, STW], f16, tag="sinM")
        sb_mask = const.tile([128, 896], f16, tag="mask")
        sb_ones = const.tile([128, 1], f16, tag="ones")
        sb_onesr = const.tile([1, 128], f16, tag="onesr")

        qoff = [0, 128, 256, 384, 512, 640]
        xt = [xtwo.tile([128, STW], f16, tag="xt", name=f"xt{hc}")
              for hc in range(16)]
        # sync queue: xt (needed first, for kv latents)
        for hc in range(16):
            nc.sync.dma_start(out=xt[hc][:], in_=xT[hc * 128:(hc + 1) * 128, :])
        # scalar queue: wakv (needed first), then waq (q latents)
        for hc in range(16):
            nc.scalar.dma_start(out=sb_wakv[hc][:],
                                in_=wakv[hc * 128:(hc + 1) * 128, :])
        for hc in range(16):
            nc.scalar.dma_start(out=sb_waq[hc][:],
                                in_=waq[hc * 128:(hc + 1) * 128, :])
        # gpsimd queue: rope tables, mask, then wqb
        nc.gpsimd.dma_start(out=sb_cosM[:], in_=cosM[:])
        nc.gpsimd.dma_start(out=sb_sinM[:], in_=sinM[:])
        nc.gpsimd.dma_start(out=sb_cos[:], in_=cosT[:])
        nc.gpsimd.dma_start(out=sb_sin[:], in_=sinT[:])
        nc.gpsimd.dma_start(out=sb_mask[:], in_=maskT[:])
        for c in range(6):
            nc.gpsimd.dma_start(out=sb_wqb[:QCH[c], c * W:(c + 1) * W],
                                in_=wqb[qoff[c]:qoff[c] + QCH[c], :])
        for c in range(2):
            nc.gpsimd.dma_start(out=sb_wkn[:, c * HPC * 64:(c + 1) * HPC * 64],
                                in_=wkn[c * 128:(c + 1) * 128, :])
            nc.gpsimd.dma_start(out=sb_wv[:, c * HPC * 128:(c + 1) * HPC * 128],
                                in_=wv[c * 128:(c + 1) * 128, :])
        nc.vector.memset(sb_ones[:], 1.0)
        nc.vector.memset(sb_onesr[:], 1.0)
        # wo loads on sync queue behind xt
        sb_wo = []
        for h in range(HPC):
            t = wop.tile([128, HID], f16, tag="wo", name=f"wo{h}")
            nc.sync.dma_start(out=t[:], in_=wo[h * 128:(h + 1) * 128, :])
            sb_wo.append(t)

        # persistent per-head tensors (my 4 heads x all tokens)
        qfT = [persist.tile([128, S], f16, tag="qf", name=f"qfT{h}") for h in range(HPC)]
        kfT = [persist.tile([128, S], f16, tag="kf", name=f"kfT{h}") for h in range(HPC)]
        VT = [persist.tile([128, 16 * VDIM], f16, tag="vh", name=f"VT{h}") for h in range(HPC)]

        # DRAM bounce buffers
        bgk_in = dramp.tile([128, 1536], f16, name="bgk_in", tag="bgk_in")
        bgk_out = dramp.tile([4 * 128, 1536], f16, name="bgk_out", tag="bgk_out")
        bgq_in = dramp.tile([128, 3072], f16, name="bgq_in", tag="bgq_in")
        bgq_out = dramp.tile([4 * 128, 3072], f16, name="bgq_out", tag="bgq_out")

        # ================= P1: latents for MY supertile =================
        sums = ps_out.tile([64, STW], f32, tag="out", name="sums")  # row0 q, row32 k

        # kv latents: ckv (2 chunk halves) + kpe (64 rows), then GKV gather
        psk = ps_big.tile([128, 1024], f32, tag="big", name="klat")
        for half in range(2):
            for hc in range(16):
                nc.tensor.matmul(
                    psk[:, half * STW:(half + 1) * STW],
                    sb_wakv[hc][:, half * 128:(half + 1) * 128],
                    xt[hc][:],
                    start=(hc == 0), stop=(hc == 15))
        kraw = rawp.tile([128, 1024], f16, tag="rawk")
        nc.vector.tensor_copy(kraw[:], psk[:])
        sqk = sqp.tile([128, 1024], f16, tag="sq")
        nc.scalar.activation(sqk[:], psk[:], AF.Square)
        for half in range(2):
            nc.tensor.matmul(sums[32:33, :], sb_ones[:, :],
                             sqk[:, half * STW:(half + 1) * STW],
                             start=(half == 0), stop=(half == 1))

        psp = ps_misc.tile([128, STW], f32, tag="misc", name="kpelat")
        for hc in range(16):
            nc.tensor.matmul(psp[0:64, :], sb_wakv[hc][:, 256:320], xt[hc][:],
                             start=(hc == 0), stop=(hc == 15))
        kperaw = rawp.tile([64, STW], f16, tag="kpe")
        nc.vector.tensor_copy(kperaw[:], psp[0:64, :])

        # rstd for k, normalize kraw
        stdk = smallp.tile([1, STW], f32, tag="stdk", bufs=1)
        nc.scalar.activation(stdk[:], sums[32:33, :], AF.Sqrt,
                             bias=EPS, scale=1.0 / KVLORA)
        rstdkf = smallp.tile([1, STW], f32, tag="rstdkf", bufs=1)
        nc.vector.reciprocal_approx_fast(out=rstdkf[:], in_=stdk[:])
        rstdk = smallp.tile([1, STW], f16, tag="rstdk", bufs=1)
        nc.vector.tensor_copy(rstdk[:], rstdkf[:])
        bck = ps_misc.tile([128, STW], f32, tag="misc", name="bck")
        nc.tensor.matmul(bck[:], sb_onesr[:], rstdk[:1, :], start=True, stop=True)
        bcks = bcp.tile([128, STW], f16, tag="bc", name="bcks")
        nc.vector.tensor_copy(bcks[:], bck[:])
        nc.vector.tensor_mul(kraw[:, 0:STW], kraw[:, 0:STW], bcks[:])
        nc.vector.tensor_mul(kraw[:, STW:1024], kraw[:, STW:1024], bcks[:])

        # shared k_pe rope on [64, 512]: rotate-half = 2 plain swapped copies
        # (sign folded into sinM rows 64:96); tables rows 64:128.
        rotk = scr1.tile([64, STW], f16, tag="rotk")
        nc.scalar.copy(out=rotk[0:32, :], in_=kperaw[32:64, :])
        nc.scalar.copy(out=rotk[32:64, :], in_=kperaw[0:32, :])
        t1k = scr1.tile([64, STW], f16, tag="t1k")
        nc.vector.tensor_mul(t1k[:], rotk[:], sb_sinM[:])
        t2k = scr1.tile([64, STW], f16, tag="t2k")
        nc.vector.tensor_mul(t2k[:], kperaw[:], sb_cosM[:])
        kpero = kperaw
        nc.vector.tensor_add(kpero[:], t1k[:], t2k[:])

        nc.gpsimd.dma_start(bgk_in[:, 0:1024], kraw[:])
        nc.gpsimd.dma_start(bgk_in[64:128, 1024:1536], kpero[:])
        nc.gpsimd.collective_compute(
            "AllGather", mybir.AluOpType.bypass, replica_groups=GROUPS,
            ins=[bgk_in.opt()], outs=[bgk_out.opt()])

        # q latents: 3 chunk-pairs, normalized pre-gather
        qraw = []
        for p in range(3):
            ps = ps_big.tile([128, 1024], f32, tag="big", name=f"qlat{p}")
            for half in range(2):
                c = 2 * p + half
                for hc in range(16):
                    nc.tensor.matmul(
                        ps[:QCH[c], half * STW:(half + 1) * STW],
                        sb_waq[hc][:, qoff[c]:qoff[c] + QCH[c]],
                        xt[hc][:],
                        start=(hc == 0), stop=(hc == 15))
            raw = qrawp.tile([128, 1024], f16, tag="rawq", name=f"rawq{p}",
                             bufs=6)
            nc.vector.tensor_copy(raw[:], ps[:])
            sq = sqp.tile([128, 1024], f16, tag="sq")
            nc.scalar.activation(sq[:], ps[:], AF.Square)
            for half in range(2):
                c = 2 * p + half
                nc.tensor.matmul(sums[0:1, :], sb_ones[:QCH[c], :],
                                 sq[:QCH[c], half * STW:(half + 1) * STW],
                                 start=(c == 0), stop=(c == 5))
            qraw.append(raw)

        # rstd for q, applied pre-gather (payload stays 512-col aligned)
        stdq = smallp.tile([1, STW], f32, tag="stdq", bufs=1)
        nc.scalar.activation(stdq[:], sums[0:1, :], AF.Sqrt,
                             bias=EPS, scale=1.0 / QLORA)
        rstdqf = smallp.tile([1, STW], f32, tag="rstdqf", bufs=1)
        nc.vector.reciprocal_approx_fast(out=rstdqf[:], in_=stdq[:])
        rstdq = smallp.tile([1, STW], f16, tag="rstdq", bufs=1)
        nc.vector.tensor_copy(rstdq[:], rstdqf[:])
        bcq = ps_misc.tile([128, STW], f32, tag="misc", name="bcq")
        nc.tensor.matmul(bcq[:], sb_onesr[:], rstdq[:1, :], start=True, stop=True)
        bcqs = bcp.tile([128, STW], f16, tag="bc", name="bcqs")
        nc.scalar.copy(out=bcqs[:], in_=bcq[:])
        for p in range(3):
            for half in range(2):
                nc.vector.tensor_mul(
                    qraw[p][:, half * STW:(half + 1) * STW],
                    qraw[p][:, half * STW:(half + 1) * STW], bcqs[:])
            nc.gpsimd.dma_start(bgq_in[:, p * 1024:(p + 1) * 1024], qraw[p][:])
        nc.gpsimd.collective_compute(
            "AllGather", mybir.AluOpType.bypass, replica_groups=GROUPS,
            ins=[bgq_in.opt()], outs=[bgq_out.opt()])

        # q_b + rope for one gathered supertile
        def qb_block(st):
            cols = slice(st * STW, (st + 1) * STW)
            gq = []
            for p in range(3):
                t = qrawp.tile([128, 1024], f16, tag="rawq",
                               name=f"gq{st}_{p}", bufs=6)
                nc.sync.dma_start(
                    t[:], bgq_out[st * 128:(st + 1) * 128,
                                  p * 1024:(p + 1) * 1024])
                gq.append(t)
            for pr in range(2):
                qra = scr1.tile([128, 2 * STW], f16, tag="qra",
                                name=f"qra{st}_{pr}", bufs=1)
                for i in range(2):
                    h = 2 * pr + i
                    psq = ps_out.tile([128, STW], f32, tag="out",
                                      name=f"psq{st}_{h}")
                    for c in range(6):
                        nc.tensor.matmul(
                            psq[:],
                            sb_wqb[:QCH[c], c * W + h * 128:c * W + (h + 1) * 128],
                            gq[c // 2][:QCH[c], (c % 2) * STW:(c % 2 + 1) * STW],
                            start=(c == 0), stop=(c == 5))
                    # nope rows go straight to qfT (no rstd mul needed)
                    nc.vector.tensor_copy(qfT[h][0:64, cols], psq[0:64, :])
                    nc.vector.tensor_copy(qra[64:128, i * STW:(i + 1) * STW],
                                          psq[64:128, :])
                # rope: rows 0:64 = qra*cosq (nope, rstd); 64:128 full rope,
                # rotate-half = 2 plain swapped copies (sign in sin table)
                for i in range(2):
                    h = 2 * pr + i
                    hs = slice(i * STW, (i + 1) * STW)
                    rotc = scr1.tile([128, STW], f16, tag="rotc",
                                     name=f"rotc{st}_{pr}_{i}", bufs=2)
                    nc.scalar.copy(out=rotc[64:96, :], in_=qra[96:128, hs])
                    nc.scalar.copy(out=rotc[96:128, :], in_=qra[64:96, hs])
                    nc.vector.tensor_mul(rotc[64:128, :], rotc[64:128, :],
                                         sb_sin[64:128, cols])
                    nc.vector.tensor_mul(qra[64:128, hs], qra[64:128, hs],
                                         sb_cos[64:128, cols])
                    nc.vector.tensor_add(qfT[h][64:128, cols],
                                         rotc[64:128, :], qra[64:128, hs])

        # ================= P1b: kn/V over all supertiles (needs GKV) =======
        for st in range(NST):
            cols = slice(st * STW, (st + 1) * STW)
            gk = [latkp.tile([128, STW], f16, tag="latk", name=f"gk{st}_{c}")
                  for c in range(2)]
            for c in range(2):
                nc.sync.dma_start(
                    gk[c][:],
                    bgk_out[st * 128:(st + 1) * 128, c * STW:(c + 1) * STW])
            # shared k_pe straight into each head's kfT rows 64:128
            for h in range(HPC):
                eng = nc.scalar if h % 2 == 0 else nc.gpsimd
                eng.dma_start(
                    out=kfT[h][64:128, cols],
                    in_=bgk_out[st * 128 + 64:(st + 1) * 128, 1024:1536])
            for hp in range(2):
                pskn = ps_out.tile([128, STW], f32, tag="out", name=f"kn{st}_{hp}")
                for c in range(2):
                    nc.tensor.matmul(
                        pskn[:],
                        sb_wkn[:, c * HPC * 64 + hp * 128:c * HPC * 64 + (hp + 1) * 128],
                        gk[c][:],
                        start=(c == 0), stop=(c == 1))
                nc.scalar.copy(out=kfT[2 * hp][0:64, cols], in_=pskn[0:64, :])
                nc.scalar.copy(out=kfT[2 * hp + 1][0:64, cols], in_=pskn[64:128, :])
            for h in range(HPC):
                psv = ps_out.tile([128, STW], f32, tag="out", name=f"psv{st}_{h}")
                for tcn in range(4):
                    for c in range(2):
                        nc.tensor.matmul(
                            psv[:, tcn * VDIM:(tcn + 1) * VDIM],
                            gk[c][:, tcn * 128:(tcn + 1) * 128],
                            sb_wv[:, c * HPC * 128 + h * 128:c * HPC * 128 + (h + 1) * 128],
                            start=(c == 0), stop=(c == 1))
                nc.vector.tensor_copy(VT[h][:, st * STW:(st + 1) * STW], psv[:])

        # ================= P2 / P3, per q-supertile ========================
        # pre-zero pt slots: qs==0 diagonal chunks use full-width mask muls
        # that must see finite values in the stale trimmed columns
        for z in range(2):
            ptz = ptp.tile([128, 1024], f16, tag="pt", name=f"ptz{z}")
            nc.vector.memset(ptz[:], 0.0)

        def norm(qs, h, ssums, accs, aouts):
            bca = ps_misc.tile([128, STW], f32, tag="misc", name=f"bca{qs}_{h}")
            nc.tensor.matmul(bca[:], sb_onesr[:], ssums[h][:1, :],
                             start=True, stop=True)
            bcas = bcp.tile([128, STW], f16, tag="bc", name=f"bcas{qs}_{h}")
            nc.scalar.copy(out=bcas[:], in_=bca[:])
            ao = aoutp.tile([128, STW], f16, tag="aout", name=f"ao{qs}_{h}")
            nc.vector.tensor_mul(ao[:], accs[h][:], bcas[:])
            aouts.append(ao)

        def p2_block(qs):
            nkc = 4 * qs + 4
            ng = nkc // 2
            accs, ssums, aouts = [], [], []
            kcs = (list(range(4 * qs, nkc)) + list(range(0, 4 * qs))
                   if qs > 0 else list(range(nkc)))
            for h in range(HPC):
                outT = ps_out.tile([128, STW], f32, tag="out", name=f"oT{qs}_{h}")
                # two partial prob-sum accumulators: even chunks chain on
                # DVE (accA), odd chunks on GpSimd (accB); the combine is
                # folded into the ssum PSUM accumulation
                accA = accp.tile([128, STW], f16, tag="acc", name=f"accA{qs}_{h}")
                accB = accp.tile([128, STW], f16, tag="acc", name=f"accB{qs}_{h}")

                stps, pts = {}, {}
                def sc(g):
                    stp = ps_big.tile([128, 1024], f32, tag="big",
                                      name=f"sc{qs}_{h}_{g}")
                    trim = [0, 0]
                    for half in range(2):
                        kc = kcs[2 * g + half]
                        j = kc - 4 * qs
                        tr = 128 * j if j > 0 else 0
                        trim[half] = tr
                        nc.tensor.matmul(
                            stp[:, half * STW + tr:(half + 1) * STW],
                            kfT[h][:, kc * 128:(kc + 1) * 128],
                            qfT[h][:, qs * STW + tr:(qs + 1) * STW],
                            start=True, stop=True)
                    pt = ptp.tile([128, 1024], f16, tag="pt")
                    if trim[0] == 0 and trim[1] == 0:
                        nc.scalar.activation(pt[:], stp[:], AF.Exp,
                                             bias=NEGC, scale=SCALE)
                    else:
                        for half in range(2):
                            tr = trim[half]
                            nc.scalar.activation(
                                pt[:, half * STW + tr:(half + 1) * STW],
                                stp[:, half * STW + tr:(half + 1) * STW],
                                AF.Exp, bias=NEGC, scale=SCALE)
                    for half in range(2):
                        kc = kcs[2 * g + half]
                        j = kc - 4 * qs
                        if j >= 0:
                            tr = 128 * j
                            if qs == 0 and j > 0:
                                # full-width mask (zeroes stale cols too)
                                ph = pt[:, half * STW:(half + 1) * STW]
                                nc.vector.tensor_mul(
                                    ph, ph,
                                    sb_mask[:, 384 - tr:896 - tr])
                            else:
                                # triangle block only
                                blk = slice(half * STW + tr,
                                            half * STW + tr + 128)
                                nc.vector.tensor_mul(
                                    pt[:, blk], pt[:, blk],
                                    sb_mask[:, 384:512])
                    pts[g] = pt

                def av(g, first):
                    pt = pts.pop(g)
                    for half in range(2):
                        idx = 2 * g + half
                        kc = kcs[idx]
                        j = kc - 4 * qs
                        tr = 128 * j if (j > 0 and qs > 0) else 0
                        ph = pt[:, half * STW + tr:(half + 1) * STW]
                        nc.tensor.matmul(outT[:, tr:STW],
                                         VT[h][:, kc * VDIM:(kc + 1) * VDIM],
                                         ph,
                                         start=(idx == 0), stop=(idx == nkc - 1))
                        ph2 = pt[:, half * STW + tr:(half + 1) * STW]
                        if idx == 0:
                            first[0] = pt[:, half * STW:(half + 1) * STW]
                        elif idx == 1:
                            if tr > 0:
                                nc.gpsimd.memset(accB[:, 0:tr], 0.0)
                                nc.gpsimd.tensor_copy(accB[:, tr:STW], ph2)
                            else:
                                nc.gpsimd.tensor_copy(
                                    accB[:], pt[:, half * STW:(half + 1) * STW])
                        elif idx == 2:
                            if tr > 0:
                                nc.vector.tensor_copy(accA[:, 0:tr],
                                                      first[0][:, 0:tr])
                            nc.vector.tensor_add(
                                accA[:, tr:STW], first[0][:, tr:STW], ph2)
                        elif idx % 2 == 0:
                            nc.vector.tensor_add(
                                accA[:, tr:STW], accA[:, tr:STW], ph2)
                        else:
                            nc.gpsimd.tensor_add(
                                accB[:, tr:STW], accB[:, tr:STW], ph2)

                first = [None]
                sc(0)
                if h >= 1:
                    norm(qs, h - 1, ssums, accs, aouts)
                for g in range(ng):
                    if g + 1 < ng:
                        sc(g + 1)
                    av(g, first)
                ssum = ps_misc.tile([1, STW], f32, tag="misc", name=f"ss{qs}_{h}")
                nc.tensor.matmul(ssum[:], sb_ones[:, :], accA[:],
                                 start=True, stop=False)
                nc.tensor.matmul(ssum[:], sb_ones[:, :], accB[:],
                                 start=False, stop=True)
                rsf = smallp.tile([1, STW], f32, tag="rsf", bufs=2)
                nc.vector.reciprocal_approx_fast(out=rsf[:], in_=ssum[:])
                rs = smallp.tile([1, STW], f16, tag="rs", bufs=4)
                nc.vector.tensor_copy(rs[:], rsf[:])
                ssums.append(rs)
                accs.append(outT)
            norm(qs, HPC - 1, ssums, accs, aouts)
            return aouts

        def p3_block(qs, aouts):
            # local partial o_proj: out^T [2048 hid, 512 tok] over my 4 heads
            rs_in = dramp.tile([NH * 128, STW], f16, tag="rs_in",
                               name=f"rsin{qs}", bufs=2)
            for hcn in range(16):
                pso = ps_out.tile([128, STW], f32, tag="out", name=f"pso{qs}_{hcn}")
                for h in range(HPC):
                    nc.tensor.matmul(
                        pso[:],
                        sb_wo[h][:, hcn * 128:(hcn + 1) * 128],
                        aouts[h][:],
                        start=(h == 0), stop=(h == HPC - 1))
                ob = waqp.tile([128, STW], f16, tag="waq", name=f"ob{qs}_{hcn}")
                if hcn % 2 == 0:
                    nc.scalar.copy(out=ob[:], in_=pso[:])
                else:
                    nc.vector.tensor_copy(ob[:], pso[:])
                deng = nc.sync if hcn % 2 == 0 else nc.gpsimd
                deng.dma_start(rs_in[hcn * 128:(hcn + 1) * 128, :], ob[:])
            rs_out = dramp.tile([STW, STW], f16, tag="rs_out",
                                name=f"rsout{qs}", bufs=2)
            nc.gpsimd.collective_compute(
                "ReduceScatter", mybir.AluOpType.add, replica_groups=GROUPS,
                ins=[rs_in.opt()], outs=[rs_out.opt()])
            nc.sync.dma_start(out[qs * STW:(qs + 1) * STW, :], rs_out[:])

        for st in range(NST):
            qb_block(st)
        for qs in range(NST):
            aouts = p2_block(qs)
            p3_block(qs, aouts)

    nc.compile()
    return nc


def _host_prep(inputs):
    f16 = np.float16
    x = np.asarray(inputs["x"], np.float32)
    q_a_w = np.asarray(inputs["q_a_w"], np.float32)
    q_a_ln = np.asarray(inputs["q_a_ln_w"], np.float32)
    q_b_w = np.asarray(inputs["q_b_w"], np.float32)
    kv_a_w = np.asarray(inputs["kv_a_w"], np.float32)
    kv_a_ln = np.asarray(inputs["kv_a_ln_w"], np.float32)
    kv_b_w = np.asarray(inputs["kv_b_w"], np.float32)
    o_w = np.asarray(inputs["o_w"], np.float32)

    perm = np.concatenate([np.arange(0, ROPE, 2), np.arange(1, ROPE, 2)])
    q_b_f = q_b_w * q_a_ln[:, None]
    kv_b_f = kv_b_w * kv_a_ln[:, None]

    # kv_a: [ckv 256 | kpe perm 64]
    wakv = np.concatenate(
        [kv_a_w[:, :KVLORA], kv_a_w[:, KVLORA:][:, perm]], axis=1).astype(f16)
    waq = q_a_w.astype(f16)

    # rope tables (transposed [dim, pos]); rotate-half sign folded into sin:
    # rows 64:96 = -sin[0:32], rows 96:128 = +sin[32:64]
    inv = 1.0 / (THETA ** (np.arange(0, ROPE, 2, dtype=np.float64) / ROPE))
    freqs = np.outer(np.arange(S, dtype=np.float64), inv)      # [S, 32]
    cos64 = np.concatenate([np.cos(freqs), np.cos(freqs)], -1).T  # [64, S]
    sin64 = np.concatenate([np.sin(freqs), np.sin(freqs)], -1).T
    cosT = np.concatenate([np.ones((64, S)), cos64], 0).astype(f16)
    sinT = np.concatenate([np.zeros((64, S)), -sin64[0:32], sin64[32:64]],
                          0).astype(f16)

    # shifted causal window: maskT[k, c] = k <= c - 384; cols 384:512 are the
    # plain [128,128] triangle k <= q'
    k_i = np.arange(128)[:, None]
    c_i = np.arange(896)[None, :]
    maskT = (k_i <= c_i - 384).astype(f16)

    in_maps = []
    for core in range(NCORES):
        b = core // 4
        j = core % 4
        heads = [HPC * j + i for i in range(HPC)]
        wqb = np.concatenate(
            [np.concatenate(
                [q_b_f[:, h * QHEAD:h * QHEAD + NOPE],
                 q_b_f[:, h * QHEAD + NOPE:(h + 1) * QHEAD][:, perm]], 1)
             for h in heads], axis=1).astype(f16)
        wkn = np.concatenate(
            [kv_b_f[:, h * (NOPE + VDIM):h * (NOPE + VDIM) + NOPE]
             for h in heads], axis=1).astype(f16)
        wv = np.concatenate(
            [kv_b_f[:, h * (NOPE + VDIM) + NOPE:(h + 1) * (NOPE + VDIM)]
             for h in heads], axis=1).astype(f16)
        wo = np.concatenate(
            [o_w[h * VDIM:(h + 1) * VDIM, :] for h in heads], axis=0).astype(f16)
        scols = slice(j * STW, (j + 1) * STW)
        in_maps.append({
            "xT": np.ascontiguousarray(x[b].T[:, scols]).astype(f16),
            "waq": waq, "wakv": wakv, "wqb": wqb, "wkn": wkn, "wv": wv,
            "wo": wo, "cosT": cosT, "sinT": sinT,
            "cosM": np.ascontiguousarray(cosT[64:128, scols]),
            "sinM": np.ascontiguousarray(sinT[64:128, scols]),
            "maskT": maskT,
        })
    return in_maps


def kernel(**inputs):
    global _PROGRAM
    _ensure_axon_hooks_shim()
    from concourse.bass_utils import run_bass_kernel_spmd

    if _PROGRAM is None:
        _PROGRAM = _build_program()
    in_maps = _host_prep(inputs)
    res = run_bass_kernel_spmd(_PROGRAM, in_maps, list(range(NCORES)))
    out = np.zeros((B, S, HID), np.float32)
    for core in range(NCORES):
        b, j = core // 4, core % 4
        r = res.results[core]["out"].astype(np.float32)   # [4*512 hid-qs, 512]
        for qs in range(NST):
            out[b][qs * STW:(qs + 1) * STW, j * STW:(j + 1) * STW] = \
                r[qs * STW:(qs + 1) * STW, :].T
    return out
